# revision 10
# baseline (speedup 1.0000x reference)
"""Trainium2 Bass kernel for a GPT-style transformer block (B=2,T=2048,C=768,H=12).

Sharding: 8 cores; core c handles batch b=c//4, query block qo=(c%4)*512.
Each core gets its batch's x feature-major [C,T], rolled so its 512 query
tokens are columns 0:512.  K/V are computed for all 2048 keys (duplicated
across the 4 cores of a batch); Q/attention/MLP only for the 512 queries.

v2 structure (vs the 351us baseline):
 - x_bf is DMA'd in four 512-token chunks and LN1 stats / V / K0 / Q are
   pipelined per chunk, so the PE starts ~6us in instead of ~30us.
 - Scores are row-tiled: head pair (2ch, 2ch+1) runs as two concurrent
   K=64 matmuls (tile_position (0,0)/(64,0)) into two PSUM banks, halving
   score time.  One exp covers both banks.  Denominators still ride the
   augmented-V ones column.
 - Softmax reciprocal stays on DVE except the last pair (Act ln/exp) so
   Wo isn't blocked by the 3.4us DVE reciprocal.
 - LN2 is folded into the FC matmul: FC runs on the *uncentered* residual
   x2 (bf16) immediately after Wo, with a rank-1 (-mu * colsum(Wfc))
   correction and the 1/std multiply applied at PSUM copyback before
   gelu.  The PE never idles (and never goes HAM-cold) at the Wo->MLP
   boundary.
 - MLP is two passes: FC -> hc (all 24 chunks in SBUF), then proj
   oc-major with per-oc output DMA, so the output store overlaps the
   last proj matmuls.
 - Act only ever uses the natural_log_exp set until the first gelu
   (squares moved to DVE/GpSimd), so ~2 table loads instead of 12.
"""
import sys

sys.path.insert(0, "/opt/trn_rl_repo")

import numpy as np
import ml_dtypes

import concourse.bass as bass
import concourse.tile as tile
from concourse import bacc, mybir
from concourse.bass_utils import run_bass_kernel_spmd

F32 = mybir.dt.float32
F32R = mybir.dt.float32r
BF16 = mybir.dt.bfloat16
AF = mybir.ActivationFunctionType
ALU = mybir.AluOpType

B, T, C, H = 2, 2048, 768, 12
HD = C // H             # 64
C4 = 4 * C              # 3072
EPS = 1e-5
NCORES = 8
TQ = (B * T) // NCORES  # 512
PC = C // 128           # 6
PC4 = C4 // 128         # 24
NT4 = T // 512          # 4
NSC = T // 128          # 16
NBIAS = (5 * C + C4) // 128  # 54
NPAIR = H // 2          # 6
FC_DELAY = 5            # kc lag between FC accumulate and gelu copyback


def _build(has_qkv_bias, has_o_bias, has_proj_bias, has_fc_bias, has_mask, reps=1):
    has_bias_any = has_qkv_bias or has_o_bias or has_proj_bias or has_fc_bias
    nc = bacc.Bacc()

    x_d = nc.dram_tensor("x_fm", [C, T], F32, kind="ExternalInput")
    xb_d = nc.dram_tensor("x_bf", [C, T], BF16, kind="ExternalInput")
    wq_d = nc.dram_tensor("wq", [C, C], BF16, kind="ExternalInput")
    wk_d = nc.dram_tensor("wk", [C, C], BF16, kind="ExternalInput")
    wv_d = nc.dram_tensor("wv", [C, C], BF16, kind="ExternalInput")
    wo_d = nc.dram_tensor("wo", [128, PC, C], BF16, kind="ExternalInput")
    wfc_d = nc.dram_tensor("wfc", [PC4, 128, PC, 128], BF16, kind="ExternalInput")
    wproj_d = nc.dram_tensor("wproj", [PC, 128, PC4, 128], BF16, kind="ExternalInput")
    wsumfc_d = nc.dram_tensor("wsumsfc", [1, C4], BF16, kind="ExternalInput")
    bias_d = nc.dram_tensor("biases", [128, NBIAS], F32, kind="ExternalInput")
    brow_d = nc.dram_tensor("bias_rows", [1, 3 * C], BF16, kind="ExternalInput")
    mask_d = nc.dram_tensor("maskb", [128, NSC], F32, kind="ExternalInput")
    out_d = nc.dram_tensor("out_fm", [C, TQ], F32, kind="ExternalOutput")

    x_pot = x_d.rearrange("(o p) t -> p o t", p=128)
    xb_pot = xb_d.rearrange("(o p) t -> p o t", p=128)
    out_pot = out_d.rearrange("(o p) t -> p o t", p=128)

    with tile.TileContext(nc) as tc:
      for _rep in range(reps):
        with tc.tile_pool(name=f"const{_rep}", bufs=1) as const, \
             tc.tile_pool(name=f"persist{_rep}", bufs=1) as persist, \
             tc.tile_pool(name=f"mlp1_{_rep}", bufs=1) as mlp1:

            # ---------------- constants (no DMA) ----------------
            ones_col_b = const.tile([128, 1], BF16)
            nc.vector.memset(ones_col_b[:], 1.0)
            ones_row_b = const.tile([1, 128], BF16)
            nc.vector.memset(ones_row_b[:], 1.0)
            ones_mat = const.tile([128, HD + 1], BF16)
            nc.vector.memset(ones_mat[:], 1.0)

            # residual tile: holds x for the queries, then x + attn_out in place
            x2 = persist.tile([128, PC, TQ], F32)
            x2b = mlp1.tile([128, PC, TQ], BF16)
            hc_all = mlp1.tile([128, PC4, TQ], BF16)
            wsumfc_sb = mlp1.tile([1, C4], BF16)

            with tc.tile_pool(name=f"ypool{_rep}", bufs=1) as ypool:
              y_sb = ypool.tile([HD + 1, H, TQ], BF16)
              y_nm2 = ypool.tile([128, PC, TQ], BF16)
              wo_sb = ypool.tile([128, PC, C], BF16)
              with tc.tile_pool(name=f"attp{_rep}", bufs=1) as attp:
                q2 = attp.tile([128, NPAIR, TQ], BF16)
                k_bf = attp.tile([128, PC, T], BF16)
                vt_aug = attp.tile([128, NSC, H * (HD + 1)], BF16)
                x_bf = attp.tile([128, PC, T], BF16)
                istd_b = attp.tile([128, T], BF16)
                istd_col = attp.tile([128, NSC], F32)
                risd_r = (attp.tile([1, T], BF16)    # sqrt(var+eps) (bias path)
                          if has_qkv_bias else None)
                wk_sb = attp.tile([128, PC, C], BF16)

                # ======== phase A: LN1 stats + V + K0 + Q, per chunk ========
                with tc.tile_pool(name=f"rtmp{_rep}", bufs=2) as rtmp, \
                     tc.tile_pool(name=f"gsc{_rep}", bufs=2) as gsc, \
                     tc.tile_pool(name=f"wcyc{_rep}", bufs=2) as wcyc, \
                     tc.tile_pool(name=f"st_ps{_rep}", bufs=1, space="PSUM") as st_ps, \
                     tc.tile_pool(name=f"p1_ps{_rep}", bufs=2, space="PSUM") as p1_ps, \
                     tc.tile_pool(name=f"vq_ps{_rep}", bufs=2, space="PSUM") as vq_ps:

                    # ---- DMAs in priority order ----
                    nc.sync.dma_start(x_bf[:, :, 0:512], xb_pot[:, :, 0:512])
                    wv_sb = wcyc.tile([128, PC, C], BF16, tag="w")
                    nc.sync.dma_start(wv_sb[:], wv_d.rearrange("(o p) m -> p o m", p=128))
                    for t4 in range(1, NT4):
                        sl = slice(t4 * 512, (t4 + 1) * 512)
                        nc.sync.dma_start(x_bf[:, :, sl], xb_pot[:, :, sl])
                    wq_sb = wcyc.tile([128, PC, C], BF16, tag="w")
                    nc.sync.dma_start(wq_sb[:], wq_d.rearrange("(o p) m -> p o m", p=128))
                    nc.sync.dma_start(wk_sb[:], wk_d.rearrange("(o p) m -> p o m", p=128))
                    nc.sync.dma_start(wsumfc_sb[:], wsumfc_d[:, :])
                    if has_bias_any:
                        bias_sb = const.tile([128, NBIAS], F32)
                        nc.sync.dma_start(bias_sb[:], bias_d[:, :])
                    if has_mask:
                        mask_sb = const.tile([128, NSC], F32)
                        nc.sync.dma_start(mask_sb[:], mask_d[:, :])
                    if has_qkv_bias:
                        brow_sb = const.tile([1, 3 * C], BF16)
                        nc.sync.dma_start(brow_sb[:], brow_d[:, :])

                    p1s = {}

                    def emit_p1(t4):
                        sl = slice(t4 * 512, (t4 + 1) * 512)
                        p1 = p1_ps.tile([1, 512], F32, tag="p1")
                        for j in range(PC):
                            nc.tensor.matmul(p1[:], ones_col_b[:], x_bf[:, j, sl],
                                             start=(j == 0), stop=(j == PC - 1))
                        p1s[t4] = p1

                    def emit_center(t4):
                        # negmu row -> broadcast -> center x_bf in place
                        sl = slice(t4 * 512, (t4 + 1) * 512)
                        negmu_c = rtmp.tile([1, 512], BF16, tag="rtb")
                        nc.vector.tensor_scalar_mul(negmu_c[:], p1s[t4][:], -1.0 / C)
                        nm_ps = st_ps.tile([128, 512], F32, tag="nm")
                        nc.tensor.matmul(nm_ps[:], ones_row_b[:], negmu_c[:],
                                         start=True, stop=True)
                        nm_sb = gsc.tile([128, 512], BF16, tag="nmsb")
                        nc.scalar.activation(nm_sb[:], nm_ps[:], AF.Copy)
                        for j in range(PC):
                            eng = nc.vector if j < 3 else nc.gpsimd
                            eng.tensor_tensor(x_bf[:, j, sl], x_bf[:, j, sl],
                                              nm_sb[:], ALU.add)

                    def emit_var_chain(t4):
                        # squares (DVE/GpSimd only) -> p2 -> istd row/col/broadcast
                        sl = slice(t4 * 512, (t4 + 1) * 512)
                        p2 = p1_ps.tile([1, 512], F32, tag="p2")
                        for j in range(PC):
                            xsq = gsc.tile([128, 512], BF16, tag="xsq")
                            eng = nc.vector if j < 3 else nc.gpsimd
                            eng.tensor_tensor(xsq[:], x_bf[:, j, sl], x_bf[:, j, sl],
                                              ALU.mult)
                            nc.tensor.matmul(p2[:], ones_col_b[:], xsq[:],
                                             start=(j == 0), stop=(j == PC - 1))
                        var_c = rtmp.tile([1, 512], F32, tag="rt")
                        nc.vector.tensor_scalar(var_c[:], p2[:], 1.0 / C, EPS,
                                                ALU.mult, ALU.add)
                        lnv_c = rtmp.tile([1, 512], F32, tag="rt")
                        nc.scalar.activation(lnv_c[:], var_c[:], AF.Ln)
                        istd_c = rtmp.tile([1, 512], F32, tag="rt")
                        nc.scalar.activation(istd_c[:], lnv_c[:], AF.Exp, scale=-0.5)
                        istd_cb = rtmp.tile([1, 512], BF16, tag="rtb")
                        nc.vector.tensor_copy(istd_cb[:], istd_c[:])
                        if has_qkv_bias:
                            nc.scalar.activation(risd_r[:, sl], lnv_c[:], AF.Exp, scale=0.5)
                        bp = st_ps.tile([128, 512], F32, tag="bp")
                        nc.tensor.matmul(bp[:], ones_row_b[:], istd_cb[:],
                                         start=True, stop=True)
                        nc.scalar.activation(istd_b[:, sl], bp[:], AF.Copy)
                        for o in range(4):
                            nc.sync.dma_start(istd_col[:, t4 * 4 + o:t4 * 4 + o + 1],
                                              istd_c[0:1, o * 128:(o + 1) * 128])

                    def emit_v(t4):
                        # V for the 4 key blocks of this chunk (token-major, aug)
                        for sc in range(4 * t4, 4 * t4 + 4):
                            ssl = slice(sc * 128, (sc + 1) * 128)
                            nc.gpsimd.memset(
                                vt_aug[:, sc, :].rearrange("p (h e) -> p h e", e=HD + 1)[:, :, HD:HD + 1],
                                1.0)
                            for half in range(2):
                                hsl = slice(half * 384, (half + 1) * 384)
                                csl = slice(2 * C + half * 384, 2 * C + (half + 1) * 384)
                                vp = vq_ps.tile([128, 512], F32, tag="pp", name="vp")[:, 0:384]
                                for j in range(PC):
                                    nc.tensor.matmul(vp[:], x_bf[:, j, ssl], wv_sb[:, j, hsl],
                                                     start=(j == 0),
                                                     stop=(j == PC - 1 and not has_qkv_bias))
                                if has_qkv_bias:
                                    nc.tensor.matmul(vp[:], risd_r[:, ssl], brow_sb[:, csl],
                                                     start=False, stop=True)
                                dst = vt_aug[:, sc, :].rearrange("p (h e) -> p h e", e=HD + 1)[
                                    :, half * 6:(half + 1) * 6, 0:HD]
                                if half == 0:
                                    nc.scalar.activation(
                                        dst, vp[:].rearrange("p (h e) -> p h e", e=HD),
                                        AF.Copy, scale=istd_col[:, sc:sc + 1])
                                else:
                                    nc.vector.tensor_scalar(
                                        dst, vp[:].rearrange("p (h e) -> p h e", e=HD),
                                        istd_col[:, sc:sc + 1], None, ALU.mult)

                    def emit_k0(t4):
                        sl = slice(t4 * 512, (t4 + 1) * 512)
                        kp = vq_ps.tile([128, 512], F32, tag="pp", name="kp")
                        for j in range(PC):
                            nc.tensor.matmul(kp[:], wk_sb[:, j, 0:128],
                                             x_bf[:, j, sl], start=(j == 0),
                                             stop=(j == PC - 1 and not has_qkv_bias))
                        if has_qkv_bias:
                            nc.tensor.matmul(kp[:], brow_sb[:, C:C + 128],
                                             risd_r[:, sl], start=False, stop=True)
                        nc.vector.tensor_tensor(k_bf[:, 0, sl], kp[:], istd_b[:, sl],
                                                ALU.mult)

                    def emit_q():
                        # queries only (chunk 0); both heads of a pair in one tile
                        for oc in range(PC):
                            osl = slice(oc * 128, (oc + 1) * 128)
                            qp = vq_ps.tile([128, 512], F32, tag="pp", name="qp")
                            for j in range(PC):
                                nc.tensor.matmul(qp[:], wq_sb[:, j, osl],
                                                 x_bf[:, j, 0:TQ], start=(j == 0),
                                                 stop=(j == PC - 1 and not has_qkv_bias))
                            if has_qkv_bias:
                                nc.tensor.matmul(qp[:], brow_sb[:, osl],
                                                 risd_r[:, 0:TQ], start=False, stop=True)
                            nc.vector.tensor_tensor(q2[:, oc, :], qp[:],
                                                    istd_b[:, 0:TQ], ALU.mult)

                    # pipelined emission: p1(t4+1) emitted once p1(t4) is consumed
                    emit_p1(0)
                    emit_center(0)
                    emit_p1(1)
                    emit_var_chain(0)
                    emit_v(0)
                    # residual / Wo loads go behind the chunk-0 critical DMAs
                    nc.sync.dma_start(x2[:], x_pot[:, :, 0:TQ])
                    nc.sync.dma_start(wo_sb[:], wo_d[:, :, :])
                    emit_center(1)
                    emit_k0(0)
                    emit_q()
                    emit_p1(2)
                    emit_var_chain(1)
                    emit_v(1)
                    emit_center(2)
                    emit_k0(1)
                    emit_p1(3)
                    emit_var_chain(2)
                    emit_v(2)
                    emit_center(3)
                    emit_k0(2)
                    emit_var_chain(3)
                    emit_v(3)
                    emit_k0(3)

                # ============ phase B: attention (K oc=1..5 interleaved) ====
                with tc.tile_pool(name=f"sc_ps{_rep}", bufs=2, space="PSUM") as sc_ps, \
                     tc.tile_pool(name=f"y_psp{_rep}", bufs=2, space="PSUM") as y_psp, \
                     tc.tile_pool(name=f"rp_ps{_rep}", bufs=1, space="PSUM") as rp_ps, \
                     tc.tile_pool(name=f"kp_ps{_rep}", bufs=1, space="PSUM") as kp_ps, \
                     tc.tile_pool(name=f"attb{_rep}", bufs=3) as attb, \
                     tc.tile_pool(name=f"recb{_rep}", bufs=2) as recb:

                    def k_chunk_gen(oc):
                        # yields after each PE matmul; copybacks on DVE
                        osl = slice(oc * 128, (oc + 1) * 128)
                        for t4 in range(NT4):
                            sl = slice(t4 * 512, (t4 + 1) * 512)
                            kp = kp_ps.tile([128, 512], F32, tag="kp")
                            for j in range(PC):
                                nc.tensor.matmul(
                                    kp[:], wk_sb[:, j, osl],
                                    x_bf[:, j, sl], start=(j == 0),
                                    stop=(j == PC - 1 and not has_qkv_bias))
                                if j < PC - 1:
                                    yield
                            if has_qkv_bias:
                                nc.tensor.matmul(
                                    kp[:], brow_sb[:, C + oc * 128:C + (oc + 1) * 128],
                                    risd_r[:, sl], start=False, stop=True)
                            nc.vector.tensor_tensor(k_bf[:, oc, sl], kp[:],
                                                    istd_b[:, sl], ALU.mult)
                            yield

                    def make_tail(ch, yps):
                        def head_tail(h, yp):
                            nc.vector.tensor_copy(y_sb[:, h, :], yp[:])
                            rp = rp_ps.tile([HD + 1, TQ], F32, tag="rp")
                            if ch == NPAIR - 1:
                                # last pair: reciprocal via Act (ln, exp(-1)) to
                                # shorten the tail latency before Wo
                                lnd = recb.tile([1, TQ], F32, tag="lnd")
                                nc.scalar.activation(lnd[:], y_sb[HD:HD + 1, h, :], AF.Ln)
                                rrow = recb.tile([1, TQ], BF16, tag="rrow")
                                nc.scalar.activation(rrow[:], lnd[:], AF.Exp, scale=-1.0)
                                nc.tensor.matmul(rp[:], ones_mat[0:1, 0:HD + 1],
                                                 rrow[:], start=True, stop=True)
                                rec = recb.tile([HD + 1, TQ], F32, tag="rec")
                                nc.vector.tensor_copy(rec[:], rp[:])
                            else:
                                nc.tensor.matmul(rp[:], ones_mat[64:65, 0:HD + 1],
                                                 y_sb[HD:HD + 1, h, :],
                                                 start=True, stop=True)
                                rec = recb.tile([HD + 1, TQ], F32, tag="rec")
                                nc.vector.reciprocal(rec[:], rp[:])
                            if h % 2 == 0:
                                nc.gpsimd.tensor_tensor(y_nm2[0:HD, h // 2, :],
                                                        y_sb[0:HD, h, :],
                                                        rec[0:HD, :], ALU.mult)
                            else:
                                ytmp = recb.tile([HD, TQ], BF16, tag="ytmp")
                                nc.gpsimd.tensor_tensor(ytmp[:], y_sb[0:HD, h, :],
                                                        rec[0:HD, :], ALU.mult)
                                nc.sync.dma_start(y_nm2[HD:128, h // 2, :], ytmp[:])

                        def tail():
                            head_tail(2 * ch, yps[0])
                            head_tail(2 * ch + 1, yps[1])
                        return tail

                    kgen = None
                    pending_tail = None
                    for ch in range(NPAIR):
                        if ch < NPAIR - 1:
                            kgen = k_chunk_gen(ch + 1)
                        yp_a = y_psp.tile([HD + 1, TQ], F32, tag="yp")
                        yp_b = y_psp.tile([HD + 1, TQ], F32, tag="yp")
                        prev_av = None
                        for sc in range(NSC):
                            sp = sc_ps.tile([128, 2, 512], F32, tag="sp")
                            nc.tensor.matmul(sp[:, 0, :],
                                             k_bf[0:64, ch, sc * 128:(sc + 1) * 128],
                                             q2[0:64, ch, :],
                                             start=True, stop=True,
                                             tile_position=(0, 0))
                            nc.tensor.matmul(sp[:, 1, :],
                                             k_bf[64:128, ch, sc * 128:(sc + 1) * 128],
                                             q2[64:128, ch, :],
                                             start=True, stop=True,
                                             tile_position=(64, 0))
                            att = attb.tile([128, 2, 512], BF16, tag="att")
                            if has_mask:
                                for i in range(2):
                                    nc.scalar.activation(att[:, i, :], sp[:, i, :], AF.Exp,
                                                         bias=mask_sb[:, sc:sc + 1])
                            else:
                                nc.scalar.activation(att[:], sp[:], AF.Exp)
                            if prev_av is not None:
                                prev_av()
                            if pending_tail is not None:
                                pending_tail()
                                pending_tail = None
                            if kgen is not None:
                                for _ in range(2):
                                    if next(kgen, "end") == "end":
                                        kgen = None
                                        break

                            def av(att=att, sc=sc, ch=ch, yp_a=yp_a, yp_b=yp_b):
                                nc.tensor.matmul(yp_a[:],
                                                 vt_aug[:, sc, 65 * 2 * ch:65 * 2 * ch + 65],
                                                 att[:, 0, :],
                                                 start=(sc == 0), stop=(sc == NSC - 1))
                                nc.tensor.matmul(yp_b[:],
                                                 vt_aug[:, sc, 65 * (2 * ch + 1):65 * (2 * ch + 1) + 65],
                                                 att[:, 1, :],
                                                 start=(sc == 0), stop=(sc == NSC - 1))
                            prev_av = av
                        prev_av()
                        pending_tail = make_tail(ch, (yp_a, yp_b))
                    pending_tail()

              # ---- phase C: Wo (pairs 0-4 first) + LN2 stats interleaved ----
              negmu2_r = mlp1.tile([1, TQ], BF16)
              istd2_cb = mlp1.tile([1, TQ], BF16)
              istd2_b = mlp1.tile([128, TQ], BF16)
              with tc.tile_pool(name=f"dtmp{_rep}", bufs=2) as dtmp:
                with tc.tile_pool(name=f"wo_ps{_rep}", bufs=1, space="PSUM") as wo_ps, \
                     tc.tile_pool(name=f"xsqp{_rep}", bufs=2) as xsqp, \
                     tc.tile_pool(name=f"d_ps{_rep}", bufs=1, space="PSUM") as d_ps:
                  p1 = d_ps.tile([1, TQ], F32, tag="p1")
                  p2 = d_ps.tile([1, TQ], F32, tag="p2")
                  wops = []
                  for oc in range(PC):
                      op = wo_ps.tile([128, TQ], F32, tag=f"op{oc}", name=f"op{oc}")
                      for hp in range(PC - 1):
                          nc.tensor.matmul(op[:], wo_sb[:, hp, oc * 128:(oc + 1) * 128],
                                           y_nm2[:, hp, :], start=(hp == 0), stop=False)
                      wops.append(op)
                  for oc in range(PC):
                      nc.tensor.matmul(wops[oc][:], wo_sb[:, PC - 1, oc * 128:(oc + 1) * 128],
                                       y_nm2[:, PC - 1, :], start=False, stop=True)
                      op = wops[oc]
                      if has_o_bias:
                          nc.scalar.activation(op[:], op[:], AF.Identity,
                                               bias=bias_sb[:, 3 * PC + oc:3 * PC + oc + 1])
                      nc.vector.tensor_tensor(x2[:, oc, :], x2[:, oc, :], op[:],
                                              ALU.add)
                      nc.vector.tensor_copy(x2b[:, oc, :], x2[:, oc, :])
                      xsqa = xsqp.tile([128, TQ], BF16, tag="xsqa")
                      nc.gpsimd.tensor_tensor(xsqa[:], x2b[:, oc, :], x2b[:, oc, :],
                                              ALU.mult)
                      nc.tensor.matmul(p1[:], ones_col_b[:], x2b[:, oc, :],
                                       start=(oc == 0), stop=(oc == PC - 1))
                      nc.tensor.matmul(p2[:], ones_col_b[:], xsqa[:],
                                       start=(oc == 0), stop=(oc == PC - 1))

                  # LN2 scalars that read p1/p2 (before d_ps closes)
                  nc.vector.tensor_scalar_mul(negmu2_r[:], p1[:], -1.0 / C)
                  msq2 = dtmp.tile([1, TQ], F32, tag="dt")
                  nc.vector.tensor_tensor(msq2[:], negmu2_r[:], negmu2_r[:], ALU.mult)
                  var2 = dtmp.tile([1, TQ], F32, tag="dt")
                  nc.vector.tensor_scalar(var2[:], p2[:], 1.0 / C, EPS,
                                          ALU.mult, ALU.add)
                  nc.vector.tensor_sub(var2[:], var2[:], msq2[:])

                lnv2 = dtmp.tile([1, TQ], F32, tag="dt")
                nc.scalar.activation(lnv2[:], var2[:], AF.Ln)
                istd2 = dtmp.tile([1, TQ], F32, tag="dt")
                nc.scalar.activation(istd2[:], lnv2[:], AF.Exp, scale=-0.5)
                nc.vector.tensor_copy(istd2_cb[:], istd2[:])

            # ============ phase D: MLP (LN2 folded into FC) ============
            with tc.tile_pool(name=f"b2_ps{_rep}", bufs=1, space="PSUM") as b2_ps, \
                 tc.tile_pool(name=f"fc_ps{_rep}", bufs=FC_DELAY + 2, space="PSUM") as fc_ps, \
                 tc.tile_pool(name=f"h_sb{_rep}", bufs=2) as h_sb, \
                 tc.tile_pool(name=f"w_sb2{_rep}", bufs=3) as w_sb2:
                  bp2 = b2_ps.tile([128, TQ], F32, tag="bp2")
                  nc.tensor.matmul(bp2[:], ones_row_b[:], istd2_cb[:],
                                   start=True, stop=True)
                  nc.scalar.activation(istd2_b[:], bp2[:], AF.Copy)

                  fps = {}

                  def finish_kc(kc):
                      fp = fps.pop(kc)
                      # rank-1 LN2 mean fold, then istd multiply + gelu
                      nc.tensor.matmul(fp[:], wsumfc_sb[:, kc * 128:(kc + 1) * 128],
                                       negmu2_r[:], start=False, stop=True)
                      hm = h_sb.tile([128, TQ], BF16, tag="hm")
                      nc.vector.tensor_tensor(hm[:], fp[:], istd2_b[:], ALU.mult)
                      if has_fc_bias:
                          nc.scalar.activation(hc_all[:, kc, :], hm[:], AF.Gelu,
                                               bias=bias_sb[:, 5 * PC + kc:5 * PC + kc + 1])
                      else:
                          nc.scalar.activation(hc_all[:, kc, :], hm[:], AF.Gelu)

                  for kc in range(PC4):
                      wfcc = w_sb2.tile([128, PC, 128], BF16, tag="wfcc")
                      nc.sync.dma_start(wfcc[:], wfc_d[kc])
                      fp = fc_ps.tile([128, TQ], F32, tag="fp")
                      for j in range(PC):
                          nc.tensor.matmul(fp[:], wfcc[:, j, :], x2b[:, j, :],
                                           start=(j == 0), stop=False)
                      fps[kc] = fp
                      if kc >= FC_DELAY:
                          finish_kc(kc - FC_DELAY)
                  for kc in range(PC4 - FC_DELAY, PC4):
                      finish_kc(kc)

            # ---- proj, oc-major, with per-oc output DMA ----
            with tc.tile_pool(name=f"pr_ps{_rep}", bufs=2, space="PSUM") as pr_ps, \
                 tc.tile_pool(name=f"wpo{_rep}", bufs=2) as wpo_pool, \
                 tc.tile_pool(name=f"outp{_rep}", bufs=2) as outp:
                  for oc in range(PC):
                      wpo = wpo_pool.tile([128, PC4, 128], BF16, tag="wpo")
                      nc.sync.dma_start(wpo[:], wproj_d[oc])
                      pr = pr_ps.tile([128, TQ], F32, tag="pr")
                      for kc in range(PC4):
                          nc.tensor.matmul(pr[:], wpo[:, kc, :], hc_all[:, kc, :],
                                           start=(kc == 0), stop=(kc == PC4 - 1))
                      if has_proj_bias:
                          nc.scalar.activation(pr[:], pr[:], AF.Identity,
                                               bias=bias_sb[:, 4 * PC + oc:4 * PC + oc + 1])
                      out_t = outp.tile([128, TQ], F32, tag="out")
                      nc.vector.tensor_tensor(out_t[:], pr[:], x2[:, oc, :],
                                              ALU.add)
                      nc.sync.dma_start(out_pot[:, oc, :], out_t[:])

    nc.compile()
    return nc


_CACHE = {}


def _get_program(flags, reps=1):
    key = (flags, reps)
    if key not in _CACHE:
        _CACHE[key] = _build(*flags, reps=reps)
    return _CACHE[key]


def kernel(**inputs) -> np.ndarray:
    x = np.asarray(inputs["x"], dtype=np.float32)
    padding_mask = np.asarray(inputs["padding_mask"])
    ln1_s = np.asarray(inputs["ln1_scale"], dtype=np.float32)
    ln1_b = np.asarray(inputs["ln1_bias"], dtype=np.float32)
    ln2_s = np.asarray(inputs["ln2_scale"], dtype=np.float32)
    ln2_b = np.asarray(inputs["ln2_bias"], dtype=np.float32)
    Wq = np.asarray(inputs["Wq"], dtype=np.float32)
    Wk = np.asarray(inputs["Wk"], dtype=np.float32)
    Wv = np.asarray(inputs["Wv"], dtype=np.float32)
    bq = np.asarray(inputs["bq"], dtype=np.float32)
    bk = np.asarray(inputs["bk"], dtype=np.float32)
    bv = np.asarray(inputs["bv"], dtype=np.float32)
    Wo = np.asarray(inputs["Wo"], dtype=np.float32)
    bo = np.asarray(inputs["bo"], dtype=np.float32)
    Wfc = np.asarray(inputs["Wfc"], dtype=np.float32)
    bfc = np.asarray(inputs["bfc"], dtype=np.float32)
    Wproj = np.asarray(inputs["Wproj"], dtype=np.float32)
    bproj = np.asarray(inputs["bproj"], dtype=np.float32)

    sc_q = 1.0 / np.sqrt(HD)
    Wq_f = Wq.transpose(1, 0, 2).reshape(C, C)
    Wk_f = Wk.transpose(1, 0, 2).reshape(C, C)
    Wv_f = Wv.transpose(1, 0, 2).reshape(C, C)
    wq_eff = (ln1_s[:, None] * Wq_f * sc_q).astype(ml_dtypes.bfloat16)
    wk_eff = (ln1_s[:, None] * Wk_f).astype(ml_dtypes.bfloat16)
    wv_eff = (ln1_s[:, None] * Wv_f).astype(ml_dtypes.bfloat16)
    bq_eff = (ln1_b @ Wq_f) * sc_q + bq.reshape(C) * sc_q
    bk_eff = ln1_b @ Wk_f + bk.reshape(C)
    bv_eff = ln1_b @ Wv_f + bv.reshape(C)
    wfc_eff = (ln2_s[:, None] * Wfc).astype(ml_dtypes.bfloat16)
    bfc_eff = ln2_b @ Wfc + bfc
    wfc_pre = np.ascontiguousarray(
        wfc_eff.reshape(PC, 128, PC4, 128).transpose(2, 1, 0, 3))
    wproj_pre = np.ascontiguousarray(
        Wproj.reshape(PC4, 128, PC, 128).transpose(2, 1, 0, 3)).astype(ml_dtypes.bfloat16)
    wo_pre = np.ascontiguousarray(
        Wo.reshape(PC, 2, HD, C).transpose(1, 2, 0, 3).reshape(128, PC, C)
    ).astype(ml_dtypes.bfloat16)

    wsumsfc = wfc_eff.astype(np.float32).sum(0).astype(ml_dtypes.bfloat16)[None, :]

    biases = np.concatenate([bq_eff, bk_eff, bv_eff, bo, bproj, bfc_eff])
    bias_pre = np.ascontiguousarray(biases.reshape(NBIAS, 128).T).astype(np.float32)
    brows = np.concatenate([bq_eff, bk_eff, bv_eff]).astype(ml_dtypes.bfloat16)[None, :]

    has_qkv_bias = bool(np.abs(np.concatenate([bq_eff, bk_eff, bv_eff])).max() > 0)
    has_o_bias = bool(np.abs(bo).max() > 0)
    has_proj_bias = bool(np.abs(bproj).max() > 0)
    has_fc_bias = bool(np.abs(bfc_eff).max() > 0)
    has_mask = bool(padding_mask.any())

    nc = _get_program((has_qkv_bias, has_o_bias, has_proj_bias, has_fc_bias, has_mask))

    shared = {
        "wq": wq_eff, "wk": wk_eff, "wv": wv_eff, "wo": wo_pre,
        "wfc": wfc_pre, "wproj": wproj_pre, "wsumsfc": wsumsfc,
        "biases": bias_pre, "bias_rows": brows,
    }
    in_maps = []
    for c in range(NCORES):
        b, qo = c // (NCORES // B), (c % (NCORES // B)) * TQ
        xr = np.roll(x[b], -qo, axis=0)
        x_fm = np.ascontiguousarray(xr.T)
        x_bf = x_fm.astype(ml_dtypes.bfloat16)
        mrow = np.roll(padding_mask[b], -qo)
        maskb = np.ascontiguousarray(
            np.where(mrow, -1e30, 0.0).astype(np.float32).reshape(NSC, 128).T)
        in_maps.append({**shared, "x_fm": x_fm, "x_bf": x_bf, "maskb": maskb})

    res = run_bass_kernel_spmd(nc, in_maps, core_ids=list(range(NCORES)))

    out = np.empty((B, T, C), dtype=np.float32)
    for c in range(NCORES):
        b, qo = c // (NCORES // B), (c % (NCORES // B)) * TQ
        out[b, qo:qo + TQ, :] = res.results[c]["out_fm"].T
    return out


# revision 13
# speedup vs baseline: 1.1845x; 1.1845x over previous
"""Trainium2 Bass kernel for a GPT-style transformer block (B=2,T=2048,C=768,H=12).

Sharding: 8 cores; core c handles batch b=c//4, query block qo=(c%4)*512.
Each core gets its batch's x feature-major [C,T], rolled so its 512 query
tokens are columns 0:512.  K/V are computed for all 2048 keys (duplicated
across the 4 cores of a batch); Q/attention/MLP only for the 512 queries.

v2 structure (vs the 351us baseline):
 - x_bf is DMA'd in four 512-token chunks and LN1 stats / V / K0 / Q are
   pipelined per chunk, so the PE starts ~6us in instead of ~30us.
 - Scores are row-tiled: head pair (2ch, 2ch+1) runs as two concurrent
   K=64 matmuls (tile_position (0,0)/(64,0)) into two PSUM banks, halving
   score time.  One exp covers both banks.  Denominators still ride the
   augmented-V ones column.
 - Softmax reciprocal stays on DVE except the last pair (Act ln/exp) so
   Wo isn't blocked by the 3.4us DVE reciprocal.
 - LN2 is folded into the FC matmul: FC runs on the *uncentered* residual
   x2 (bf16) immediately after Wo, with a rank-1 (-mu * colsum(Wfc))
   correction and the 1/std multiply applied at PSUM copyback before
   gelu.  The PE never idles (and never goes HAM-cold) at the Wo->MLP
   boundary.
 - MLP is two passes: FC -> hc (all 24 chunks in SBUF), then proj
   oc-major with per-oc output DMA, so the output store overlaps the
   last proj matmuls.
 - Act only ever uses the natural_log_exp set until the first gelu
   (squares moved to DVE/GpSimd), so ~2 table loads instead of 12.
"""
import sys

sys.path.insert(0, "/opt/trn_rl_repo")

import numpy as np
import ml_dtypes

import concourse.bass as bass
import concourse.tile as tile
from concourse import bacc, mybir
from concourse.bass_utils import run_bass_kernel_spmd

F32 = mybir.dt.float32
F32R = mybir.dt.float32r
BF16 = mybir.dt.bfloat16
AF = mybir.ActivationFunctionType
ALU = mybir.AluOpType

B, T, C, H = 2, 2048, 768, 12
HD = C // H             # 64
C4 = 4 * C              # 3072
EPS = 1e-5
NCORES = 8
TQ = (B * T) // NCORES  # 512
PC = C // 128           # 6
PC4 = C4 // 128         # 24
NT4 = T // 512          # 4
NSC = T // 128          # 16
NBIAS = (5 * C + C4) // 128  # 54
NPAIR = H // 2          # 6
FC_DELAY = 5            # kc lag between FC accumulate and gelu copyback


def _build(has_qkv_bias, has_o_bias, has_proj_bias, has_fc_bias, has_mask, reps=1):
    has_bias_any = has_qkv_bias or has_o_bias or has_proj_bias or has_fc_bias
    nc = bacc.Bacc()

    x_d = nc.dram_tensor("x_fm", [C, T], F32, kind="ExternalInput")
    xb_d = nc.dram_tensor("x_bf", [C, T], BF16, kind="ExternalInput")
    wq_d = nc.dram_tensor("wq", [C, C], BF16, kind="ExternalInput")
    wk_d = nc.dram_tensor("wk", [C, C], BF16, kind="ExternalInput")
    wv_d = nc.dram_tensor("wv", [C, C], BF16, kind="ExternalInput")
    wo_d = nc.dram_tensor("wo", [128, PC, C], BF16, kind="ExternalInput")
    wfc_d = nc.dram_tensor("wfc", [PC4, 128, PC, 128], BF16, kind="ExternalInput")
    wproj_d = nc.dram_tensor("wproj", [PC, 128, PC4, 128], BF16, kind="ExternalInput")
    wsumfc_d = nc.dram_tensor("wsumsfc", [1, C4], BF16, kind="ExternalInput")
    bias_d = nc.dram_tensor("biases", [128, NBIAS], F32, kind="ExternalInput")
    brow_d = nc.dram_tensor("bias_rows", [1, 3 * C], BF16, kind="ExternalInput")
    mask_d = nc.dram_tensor("maskb", [128, NSC], F32, kind="ExternalInput")
    out_d = nc.dram_tensor("out_fm", [C, TQ], F32, kind="ExternalOutput")

    x_pot = x_d.rearrange("(o p) t -> p o t", p=128)
    xb_pot = xb_d.rearrange("(o p) t -> p o t", p=128)
    out_pot = out_d.rearrange("(o p) t -> p o t", p=128)

    with tile.TileContext(nc) as tc:
      for _rep in range(reps):
        with tc.tile_pool(name=f"const{_rep}", bufs=1) as const, \
             tc.tile_pool(name=f"persist{_rep}", bufs=1) as persist, \
             tc.tile_pool(name=f"mlp1_{_rep}", bufs=1) as mlp1:

            # ---------------- constants (no DMA) ----------------
            ones_col_b = const.tile([128, 1], BF16)
            nc.vector.memset(ones_col_b[:], 1.0)
            ones_row_b = const.tile([1, 128], BF16)
            nc.vector.memset(ones_row_b[:], 1.0)
            ones_mat = const.tile([128, HD + 1], BF16)
            nc.vector.memset(ones_mat[:], 1.0)

            # residual tile: holds x for the queries, then x + attn_out in place
            x2 = persist.tile([128, PC, TQ], F32)
            x2b = mlp1.tile([128, PC, TQ], BF16)
            hc_all = mlp1.tile([128, PC4, TQ], BF16)
            wsumfc_sb = mlp1.tile([1, C4], BF16)

            with tc.tile_pool(name=f"ypool{_rep}", bufs=1) as ypool:
              y_sb = ypool.tile([HD + 1, H, TQ], BF16)
              y_nm2 = ypool.tile([128, PC, TQ], BF16)
              wo_sb = ypool.tile([128, PC, C], BF16)
              with tc.tile_pool(name=f"attp{_rep}", bufs=1) as attp:
                q2 = attp.tile([128, NPAIR, TQ], BF16)
                k_bf = attp.tile([128, PC, T], BF16)
                vt_aug = attp.tile([128, NSC, H * (HD + 1)], BF16)
                x_bf = attp.tile([128, PC, T], BF16)
                istd_b = attp.tile([128, T], BF16)
                istd_col = attp.tile([128, NSC], F32)
                risd_r = (attp.tile([1, T], BF16)    # sqrt(var+eps) (bias path)
                          if has_qkv_bias else None)
                wk_sb = attp.tile([128, PC, C], BF16)

                # ======== phase A: LN1 stats + V + K0 + Q, per chunk ========
                with tc.tile_pool(name=f"rtmp{_rep}", bufs=2) as rtmp, \
                     tc.tile_pool(name=f"gsc{_rep}", bufs=2) as gsc, \
                     tc.tile_pool(name=f"wcyc{_rep}", bufs=2) as wcyc, \
                     tc.tile_pool(name=f"st_ps{_rep}", bufs=1, space="PSUM") as st_ps, \
                     tc.tile_pool(name=f"p1_ps{_rep}", bufs=2, space="PSUM") as p1_ps, \
                     tc.tile_pool(name=f"vq_ps{_rep}", bufs=2, space="PSUM") as vq_ps:

                    # ---- DMAs in priority order ----
                    nc.sync.dma_start(x_bf[:, :, 0:512], xb_pot[:, :, 0:512])
                    wv_sb = wcyc.tile([128, PC, C], BF16, tag="w")
                    nc.sync.dma_start(wv_sb[:], wv_d.rearrange("(o p) m -> p o m", p=128))
                    for t4 in range(1, NT4):
                        sl = slice(t4 * 512, (t4 + 1) * 512)
                        nc.sync.dma_start(x_bf[:, :, sl], xb_pot[:, :, sl])
                    wq_sb = wcyc.tile([128, PC, C], BF16, tag="w")
                    nc.sync.dma_start(wq_sb[:], wq_d.rearrange("(o p) m -> p o m", p=128))
                    nc.sync.dma_start(wk_sb[:], wk_d.rearrange("(o p) m -> p o m", p=128))
                    nc.sync.dma_start(wsumfc_sb[:], wsumfc_d[:, :])
                    if has_bias_any:
                        bias_sb = const.tile([128, NBIAS], F32)
                        nc.sync.dma_start(bias_sb[:], bias_d[:, :])
                    if has_mask:
                        mask_sb = const.tile([128, NSC], F32)
                        nc.sync.dma_start(mask_sb[:], mask_d[:, :])
                    if has_qkv_bias:
                        brow_sb = const.tile([1, 3 * C], BF16)
                        nc.sync.dma_start(brow_sb[:], brow_d[:, :])

                    p1s = {}

                    def emit_p1(t4):
                        sl = slice(t4 * 512, (t4 + 1) * 512)
                        p1 = p1_ps.tile([1, 512], F32, tag="p1")
                        for j in range(PC):
                            nc.tensor.matmul(p1[:], ones_col_b[:], x_bf[:, j, sl],
                                             start=(j == 0), stop=(j == PC - 1))
                        p1s[t4] = p1

                    def emit_center(t4):
                        # negmu row -> broadcast -> center x_bf in place
                        sl = slice(t4 * 512, (t4 + 1) * 512)
                        negmu_c = rtmp.tile([1, 512], BF16, tag="rtb")
                        nc.vector.tensor_scalar_mul(negmu_c[:], p1s[t4][:], -1.0 / C)
                        nm_ps = st_ps.tile([128, 512], F32, tag="nm")
                        nc.tensor.matmul(nm_ps[:], ones_row_b[:], negmu_c[:],
                                         start=True, stop=True)
                        nm_sb = gsc.tile([128, 512], BF16, tag="nmsb")
                        nc.scalar.activation(nm_sb[:], nm_ps[:], AF.Copy)
                        for j in range(PC):
                            eng = nc.vector if j < 3 else nc.gpsimd
                            eng.tensor_tensor(x_bf[:, j, sl], x_bf[:, j, sl],
                                              nm_sb[:], ALU.add)

                    def emit_var_chain(t4):
                        # squares (DVE/GpSimd only) -> p2 -> istd row/col/broadcast
                        sl = slice(t4 * 512, (t4 + 1) * 512)
                        p2 = p1_ps.tile([1, 512], F32, tag="p2")
                        for j in range(PC):
                            xsq = gsc.tile([128, 512], BF16, tag="xsq")
                            eng = nc.vector if j < 3 else nc.gpsimd
                            eng.tensor_tensor(xsq[:], x_bf[:, j, sl], x_bf[:, j, sl],
                                              ALU.mult)
                            nc.tensor.matmul(p2[:], ones_col_b[:], xsq[:],
                                             start=(j == 0), stop=(j == PC - 1))
                        var_c = rtmp.tile([1, 512], F32, tag="rt")
                        nc.vector.tensor_scalar(var_c[:], p2[:], 1.0 / C, EPS,
                                                ALU.mult, ALU.add)
                        # sqrt on Act (single table set); reciprocal on DVE in
                        # column form (recip cost scales with the free dim, so
                        # [128,4] is ~free where a [1,512] row costs 3.4us)
                        rsd_c = rtmp.tile([1, 512], F32, tag="rt")
                        nc.scalar.activation(rsd_c[:], var_c[:], AF.Sqrt)
                        if has_qkv_bias:
                            nc.vector.tensor_copy(risd_r[:, sl], rsd_c[:])
                        rsd_col = rtmp.tile([128, 4], F32, tag="rtc")
                        for o in range(4):
                            nc.sync.dma_start(rsd_col[:, o:o + 1],
                                              rsd_c[0:1, o * 128:(o + 1) * 128])
                        nc.vector.reciprocal(istd_col[:, t4 * 4:t4 * 4 + 4], rsd_col[:])
                        istd_c = rtmp.tile([1, 512], F32, tag="rt")
                        for o in range(4):
                            nc.sync.dma_start(istd_c[0:1, o * 128:(o + 1) * 128],
                                              istd_col[:, t4 * 4 + o:t4 * 4 + o + 1])
                        istd_cb = rtmp.tile([1, 512], BF16, tag="rtb")
                        nc.vector.tensor_copy(istd_cb[:], istd_c[:])
                        bp = st_ps.tile([128, 512], F32, tag="bp")
                        nc.tensor.matmul(bp[:], ones_row_b[:], istd_cb[:],
                                         start=True, stop=True)
                        nc.scalar.activation(istd_b[:, sl], bp[:], AF.Copy)

                    def emit_v(t4):
                        # V for the 4 key blocks of this chunk (token-major, aug)
                        for sc in range(4 * t4, 4 * t4 + 4):
                            ssl = slice(sc * 128, (sc + 1) * 128)
                            nc.gpsimd.memset(
                                vt_aug[:, sc, :].rearrange("p (h e) -> p h e", e=HD + 1)[:, :, HD:HD + 1],
                                1.0)
                            for half in range(2):
                                hsl = slice(half * 384, (half + 1) * 384)
                                csl = slice(2 * C + half * 384, 2 * C + (half + 1) * 384)
                                vp = vq_ps.tile([128, 512], F32, tag="pp", name="vp")[:, 0:384]
                                for j in range(PC):
                                    nc.tensor.matmul(vp[:], x_bf[:, j, ssl], wv_sb[:, j, hsl],
                                                     start=(j == 0),
                                                     stop=(j == PC - 1 and not has_qkv_bias))
                                if has_qkv_bias:
                                    nc.tensor.matmul(vp[:], risd_r[:, ssl], brow_sb[:, csl],
                                                     start=False, stop=True)
                                dst = vt_aug[:, sc, :].rearrange("p (h e) -> p h e", e=HD + 1)[
                                    :, half * 6:(half + 1) * 6, 0:HD]
                                if half == 0:
                                    nc.scalar.activation(
                                        dst, vp[:].rearrange("p (h e) -> p h e", e=HD),
                                        AF.Copy, scale=istd_col[:, sc:sc + 1])
                                else:
                                    nc.vector.tensor_scalar(
                                        dst, vp[:].rearrange("p (h e) -> p h e", e=HD),
                                        istd_col[:, sc:sc + 1], None, ALU.mult)

                    def emit_k0(t4):
                        sl = slice(t4 * 512, (t4 + 1) * 512)
                        kp = vq_ps.tile([128, 512], F32, tag="pp", name="kp")
                        for j in range(PC):
                            nc.tensor.matmul(kp[:], wk_sb[:, j, 0:128],
                                             x_bf[:, j, sl], start=(j == 0),
                                             stop=(j == PC - 1 and not has_qkv_bias))
                        if has_qkv_bias:
                            nc.tensor.matmul(kp[:], brow_sb[:, C:C + 128],
                                             risd_r[:, sl], start=False, stop=True)
                        nc.vector.tensor_tensor(k_bf[:, 0, sl], kp[:], istd_b[:, sl],
                                                ALU.mult)

                    def emit_q():
                        # queries only (chunk 0); both heads of a pair in one tile
                        for oc in range(PC):
                            osl = slice(oc * 128, (oc + 1) * 128)
                            qp = vq_ps.tile([128, 512], F32, tag="pp", name="qp")
                            for j in range(PC):
                                nc.tensor.matmul(qp[:], wq_sb[:, j, osl],
                                                 x_bf[:, j, 0:TQ], start=(j == 0),
                                                 stop=(j == PC - 1 and not has_qkv_bias))
                            if has_qkv_bias:
                                nc.tensor.matmul(qp[:], brow_sb[:, osl],
                                                 risd_r[:, 0:TQ], start=False, stop=True)
                            nc.vector.tensor_tensor(q2[:, oc, :], qp[:],
                                                    istd_b[:, 0:TQ], ALU.mult)

                    # pipelined emission: p1(t4+1) emitted once p1(t4) is consumed
                    emit_p1(0)
                    emit_center(0)
                    emit_p1(1)
                    emit_var_chain(0)
                    emit_v(0)
                    # residual / Wo loads go behind the chunk-0 critical DMAs
                    nc.sync.dma_start(x2[:], x_pot[:, :, 0:TQ])
                    nc.sync.dma_start(wo_sb[:], wo_d[:, :, :])
                    emit_center(1)
                    emit_k0(0)
                    emit_q()
                    emit_p1(2)
                    emit_var_chain(1)
                    emit_v(1)
                    emit_center(2)
                    emit_k0(1)
                    emit_p1(3)
                    emit_var_chain(2)
                    emit_v(2)
                    emit_center(3)
                    emit_k0(2)
                    emit_var_chain(3)
                    emit_v(3)
                    emit_k0(3)

                # ============ phase B: attention (K oc=1..5 interleaved) ====
                with tc.tile_pool(name=f"sc_ps{_rep}", bufs=2, space="PSUM") as sc_ps, \
                     tc.tile_pool(name=f"y_psp{_rep}", bufs=2, space="PSUM") as y_psp, \
                     tc.tile_pool(name=f"rp_ps{_rep}", bufs=1, space="PSUM") as rp_ps, \
                     tc.tile_pool(name=f"kp_ps{_rep}", bufs=1, space="PSUM") as kp_ps, \
                     tc.tile_pool(name=f"attb{_rep}", bufs=3) as attb, \
                     tc.tile_pool(name=f"recb{_rep}", bufs=2) as recb:

                    def k_chunk_gen(oc):
                        # yields after each PE matmul; copybacks on DVE
                        osl = slice(oc * 128, (oc + 1) * 128)
                        for t4 in range(NT4):
                            sl = slice(t4 * 512, (t4 + 1) * 512)
                            kp = kp_ps.tile([128, 512], F32, tag="kp")
                            for j in range(PC):
                                nc.tensor.matmul(
                                    kp[:], wk_sb[:, j, osl],
                                    x_bf[:, j, sl], start=(j == 0),
                                    stop=(j == PC - 1 and not has_qkv_bias))
                                if j < PC - 1:
                                    yield
                            if has_qkv_bias:
                                nc.tensor.matmul(
                                    kp[:], brow_sb[:, C + oc * 128:C + (oc + 1) * 128],
                                    risd_r[:, sl], start=False, stop=True)
                            nc.vector.tensor_tensor(k_bf[:, oc, sl], kp[:],
                                                    istd_b[:, sl], ALU.mult)
                            yield

                    def make_tail(ch, yps):
                        def head_tail(h, yp):
                            nc.vector.tensor_copy(y_sb[:, h, :], yp[:])
                            rp = rp_ps.tile([HD + 1, TQ], F32, tag="rp")
                            nc.tensor.matmul(rp[:], ones_mat[64:65, 0:HD + 1],
                                             y_sb[HD:HD + 1, h, :],
                                             start=True, stop=True)
                            rec = recb.tile([HD + 1, TQ], F32, tag="rec")
                            nc.vector.reciprocal(rec[:], rp[:])
                            if h % 2 == 0:
                                nc.gpsimd.tensor_tensor(y_nm2[0:HD, h // 2, :],
                                                        y_sb[0:HD, h, :],
                                                        rec[0:HD, :], ALU.mult)
                            else:
                                ytmp = recb.tile([HD, TQ], BF16, tag="ytmp")
                                nc.gpsimd.tensor_tensor(ytmp[:], y_sb[0:HD, h, :],
                                                        rec[0:HD, :], ALU.mult)
                                nc.sync.dma_start(y_nm2[HD:128, h // 2, :], ytmp[:])

                        def tail():
                            head_tail(2 * ch, yps[0])
                            head_tail(2 * ch + 1, yps[1])
                        return tail

                    kgen = None
                    pending_tail = None
                    for ch in range(NPAIR):
                        if ch < NPAIR - 1:
                            kgen = k_chunk_gen(ch + 1)
                        yp_a = y_psp.tile([HD + 1, TQ], F32, tag="yp")
                        yp_b = y_psp.tile([HD + 1, TQ], F32, tag="yp")
                        prev_av = None
                        for sc in range(NSC):
                            sp = sc_ps.tile([128, 2, 512], F32, tag="sp")
                            nc.tensor.matmul(sp[:, 0, :],
                                             k_bf[0:64, ch, sc * 128:(sc + 1) * 128],
                                             q2[0:64, ch, :],
                                             start=True, stop=True,
                                             tile_position=(0, 0))
                            nc.tensor.matmul(sp[:, 1, :],
                                             k_bf[64:128, ch, sc * 128:(sc + 1) * 128],
                                             q2[64:128, ch, :],
                                             start=True, stop=True,
                                             tile_position=(64, 0))
                            att = attb.tile([128, 2, 512], BF16, tag="att")
                            if has_mask:
                                for i in range(2):
                                    nc.scalar.activation(att[:, i, :], sp[:, i, :], AF.Exp,
                                                         bias=mask_sb[:, sc:sc + 1])
                            else:
                                nc.scalar.activation(att[:], sp[:], AF.Exp)
                            if prev_av is not None:
                                prev_av()
                            if pending_tail is not None:
                                pending_tail()
                                pending_tail = None
                            if kgen is not None:
                                for _ in range(2 if sc % 2 == 0 else 1):
                                    if next(kgen, "end") == "end":
                                        kgen = None
                                        break

                            def av(att=att, sc=sc, ch=ch, yp_a=yp_a, yp_b=yp_b):
                                nc.tensor.matmul(yp_a[:],
                                                 vt_aug[:, sc, 65 * 2 * ch:65 * 2 * ch + 65],
                                                 att[:, 0, :],
                                                 start=(sc == 0), stop=(sc == NSC - 1))
                                nc.tensor.matmul(yp_b[:],
                                                 vt_aug[:, sc, 65 * (2 * ch + 1):65 * (2 * ch + 1) + 65],
                                                 att[:, 1, :],
                                                 start=(sc == 0), stop=(sc == NSC - 1))
                            prev_av = av
                        prev_av()
                        pending_tail = make_tail(ch, (yp_a, yp_b))
                    pending_tail()

              # ---- phase C: Wo (pairs 0-4 first) + LN2 stats interleaved ----
              negmu2_r = mlp1.tile([1, TQ], BF16)
              istd2_cb = mlp1.tile([1, TQ], BF16)
              istd2_b = mlp1.tile([128, TQ], BF16)
              with tc.tile_pool(name=f"dtmp{_rep}", bufs=2) as dtmp:
                with tc.tile_pool(name=f"wo_ps{_rep}", bufs=1, space="PSUM") as wo_ps, \
                     tc.tile_pool(name=f"xsqp{_rep}", bufs=2) as xsqp, \
                     tc.tile_pool(name=f"d_ps{_rep}", bufs=1, space="PSUM") as d_ps:
                  p1 = d_ps.tile([1, TQ], F32, tag="p1")
                  p2 = d_ps.tile([1, TQ], F32, tag="p2")
                  wops = []
                  for oc in range(PC):
                      op = wo_ps.tile([128, TQ], F32, tag=f"op{oc}", name=f"op{oc}")
                      for hp in range(PC - 1):
                          nc.tensor.matmul(op[:], wo_sb[:, hp, oc * 128:(oc + 1) * 128],
                                           y_nm2[:, hp, :], start=(hp == 0), stop=False)
                      wops.append(op)
                  for oc in range(PC):
                      nc.tensor.matmul(wops[oc][:], wo_sb[:, PC - 1, oc * 128:(oc + 1) * 128],
                                       y_nm2[:, PC - 1, :], start=False, stop=True)
                      op = wops[oc]
                      if has_o_bias:
                          nc.scalar.activation(op[:], op[:], AF.Identity,
                                               bias=bias_sb[:, 3 * PC + oc:3 * PC + oc + 1])
                      nc.vector.tensor_tensor(x2[:, oc, :], x2[:, oc, :], op[:],
                                              ALU.add)
                      nc.vector.tensor_copy(x2b[:, oc, :], x2[:, oc, :])
                      xsqa = xsqp.tile([128, TQ], BF16, tag="xsqa")
                      nc.gpsimd.tensor_tensor(xsqa[:], x2b[:, oc, :], x2b[:, oc, :],
                                              ALU.mult)
                      nc.tensor.matmul(p1[:], ones_col_b[:], x2b[:, oc, :],
                                       start=(oc == 0), stop=(oc == PC - 1))
                      nc.tensor.matmul(p2[:], ones_col_b[:], xsqa[:],
                                       start=(oc == 0), stop=(oc == PC - 1))

                  # LN2 scalars that read p1/p2 (before d_ps closes)
                  nc.vector.tensor_scalar_mul(negmu2_r[:], p1[:], -1.0 / C)
                  msq2 = dtmp.tile([1, TQ], F32, tag="dt")
                  nc.vector.tensor_tensor(msq2[:], negmu2_r[:], negmu2_r[:], ALU.mult)
                  var2 = dtmp.tile([1, TQ], F32, tag="dt")
                  nc.vector.tensor_scalar(var2[:], p2[:], 1.0 / C, EPS,
                                          ALU.mult, ALU.add)
                  nc.vector.tensor_sub(var2[:], var2[:], msq2[:])

                rsd2 = dtmp.tile([1, TQ], F32, tag="dt")
                nc.scalar.activation(rsd2[:], var2[:], AF.Sqrt)
                rsd2_col = dtmp.tile([128, 4], F32, tag="dtc")
                for o in range(4):
                    nc.sync.dma_start(rsd2_col[:, o:o + 1],
                                      rsd2[0:1, o * 128:(o + 1) * 128])
                istd2_col = dtmp.tile([128, 4], F32, tag="dtc")
                nc.vector.reciprocal(istd2_col[:], rsd2_col[:])
                istd2 = dtmp.tile([1, TQ], F32, tag="dt")
                for o in range(4):
                    nc.sync.dma_start(istd2[0:1, o * 128:(o + 1) * 128],
                                      istd2_col[:, o:o + 1])
                nc.vector.tensor_copy(istd2_cb[:], istd2[:])

            # ============ phase D: MLP (LN2 folded into FC) ============
            with tc.tile_pool(name=f"wpo{_rep}", bufs=2) as wpo_pool:
              wpos = []

              def prefetch_wpo():
                  wpo = wpo_pool.tile([128, PC4, 128], BF16, tag="wpo")
                  nc.sync.dma_start(wpo[:], wproj_d[len(wpos)])
                  wpos.append(wpo)

              with tc.tile_pool(name=f"b2_ps{_rep}", bufs=1, space="PSUM") as b2_ps, \
                   tc.tile_pool(name=f"fc_ps{_rep}", bufs=FC_DELAY + 2, space="PSUM") as fc_ps, \
                   tc.tile_pool(name=f"h_sb{_rep}", bufs=2) as h_sb, \
                   tc.tile_pool(name=f"w_sb2{_rep}", bufs=3) as w_sb2:
                    bp2 = b2_ps.tile([128, TQ], F32, tag="bp2")
                    nc.tensor.matmul(bp2[:], ones_row_b[:], istd2_cb[:],
                                     start=True, stop=True)
                    nc.scalar.activation(istd2_b[:], bp2[:], AF.Copy)

                    fps = {}

                    def finish_kc(kc):
                        fp = fps.pop(kc)
                        # rank-1 LN2 mean fold, then istd multiply + gelu
                        nc.tensor.matmul(fp[:], wsumfc_sb[:, kc * 128:(kc + 1) * 128],
                                         negmu2_r[:], start=False, stop=True)
                        hm = h_sb.tile([128, TQ], BF16, tag="hm")
                        nc.vector.tensor_tensor(hm[:], fp[:], istd2_b[:], ALU.mult)
                        if has_fc_bias:
                            nc.scalar.activation(hc_all[:, kc, :], hm[:], AF.Gelu,
                                                 bias=bias_sb[:, 5 * PC + kc:5 * PC + kc + 1])
                        else:
                            nc.scalar.activation(hc_all[:, kc, :], hm[:], AF.Gelu)

                    for kc in range(PC4):
                        wfcc = w_sb2.tile([128, PC, 128], BF16, tag="wfcc")
                        nc.sync.dma_start(wfcc[:], wfc_d[kc])
                        fp = fc_ps.tile([128, TQ], F32, tag="fp")
                        for j in range(PC):
                            nc.tensor.matmul(fp[:], wfcc[:, j, :], x2b[:, j, :],
                                             start=(j == 0), stop=False)
                        fps[kc] = fp
                        if kc >= FC_DELAY:
                            finish_kc(kc - FC_DELAY)
                        if kc in (16, 20):
                            prefetch_wpo()
                    for kc in range(PC4 - FC_DELAY, PC4):
                        finish_kc(kc)

              # ---- proj, oc-major, with per-oc output DMA ----
              with tc.tile_pool(name=f"pr_ps{_rep}", bufs=2, space="PSUM") as pr_ps, \
                   tc.tile_pool(name=f"outp{_rep}", bufs=2) as outp:
                    for oc in range(PC):
                        if len(wpos) < PC:
                            prefetch_wpo()
                        wpo = wpos[oc]
                        pr = pr_ps.tile([128, TQ], F32, tag="pr")
                        for kc in range(PC4):
                            nc.tensor.matmul(pr[:], wpo[:, kc, :], hc_all[:, kc, :],
                                             start=(kc == 0), stop=(kc == PC4 - 1))
                        if has_proj_bias:
                            nc.scalar.activation(pr[:], pr[:], AF.Identity,
                                                 bias=bias_sb[:, 4 * PC + oc:4 * PC + oc + 1])
                        out_t = outp.tile([128, TQ], F32, tag="out")
                        nc.vector.tensor_tensor(out_t[:], pr[:], x2[:, oc, :],
                                                ALU.add)
                        nc.sync.dma_start(out_pot[:, oc, :], out_t[:])

    nc.compile()
    return nc


_CACHE = {}


def _get_program(flags, reps=1):
    key = (flags, reps)
    if key not in _CACHE:
        _CACHE[key] = _build(*flags, reps=reps)
    return _CACHE[key]


def kernel(**inputs) -> np.ndarray:
    x = np.asarray(inputs["x"], dtype=np.float32)
    padding_mask = np.asarray(inputs["padding_mask"])
    ln1_s = np.asarray(inputs["ln1_scale"], dtype=np.float32)
    ln1_b = np.asarray(inputs["ln1_bias"], dtype=np.float32)
    ln2_s = np.asarray(inputs["ln2_scale"], dtype=np.float32)
    ln2_b = np.asarray(inputs["ln2_bias"], dtype=np.float32)
    Wq = np.asarray(inputs["Wq"], dtype=np.float32)
    Wk = np.asarray(inputs["Wk"], dtype=np.float32)
    Wv = np.asarray(inputs["Wv"], dtype=np.float32)
    bq = np.asarray(inputs["bq"], dtype=np.float32)
    bk = np.asarray(inputs["bk"], dtype=np.float32)
    bv = np.asarray(inputs["bv"], dtype=np.float32)
    Wo = np.asarray(inputs["Wo"], dtype=np.float32)
    bo = np.asarray(inputs["bo"], dtype=np.float32)
    Wfc = np.asarray(inputs["Wfc"], dtype=np.float32)
    bfc = np.asarray(inputs["bfc"], dtype=np.float32)
    Wproj = np.asarray(inputs["Wproj"], dtype=np.float32)
    bproj = np.asarray(inputs["bproj"], dtype=np.float32)

    sc_q = 1.0 / np.sqrt(HD)
    Wq_f = Wq.transpose(1, 0, 2).reshape(C, C)
    Wk_f = Wk.transpose(1, 0, 2).reshape(C, C)
    Wv_f = Wv.transpose(1, 0, 2).reshape(C, C)
    wq_eff = (ln1_s[:, None] * Wq_f * sc_q).astype(ml_dtypes.bfloat16)
    wk_eff = (ln1_s[:, None] * Wk_f).astype(ml_dtypes.bfloat16)
    wv_eff = (ln1_s[:, None] * Wv_f).astype(ml_dtypes.bfloat16)
    bq_eff = (ln1_b @ Wq_f) * sc_q + bq.reshape(C) * sc_q
    bk_eff = ln1_b @ Wk_f + bk.reshape(C)
    bv_eff = ln1_b @ Wv_f + bv.reshape(C)
    wfc_eff = (ln2_s[:, None] * Wfc).astype(ml_dtypes.bfloat16)
    bfc_eff = ln2_b @ Wfc + bfc
    wfc_pre = np.ascontiguousarray(
        wfc_eff.reshape(PC, 128, PC4, 128).transpose(2, 1, 0, 3))
    wproj_pre = np.ascontiguousarray(
        Wproj.reshape(PC4, 128, PC, 128).transpose(2, 1, 0, 3)).astype(ml_dtypes.bfloat16)
    wo_pre = np.ascontiguousarray(
        Wo.reshape(PC, 2, HD, C).transpose(1, 2, 0, 3).reshape(128, PC, C)
    ).astype(ml_dtypes.bfloat16)

    wsumsfc = wfc_eff.astype(np.float32).sum(0).astype(ml_dtypes.bfloat16)[None, :]

    biases = np.concatenate([bq_eff, bk_eff, bv_eff, bo, bproj, bfc_eff])
    bias_pre = np.ascontiguousarray(biases.reshape(NBIAS, 128).T).astype(np.float32)
    brows = np.concatenate([bq_eff, bk_eff, bv_eff]).astype(ml_dtypes.bfloat16)[None, :]

    has_qkv_bias = bool(np.abs(np.concatenate([bq_eff, bk_eff, bv_eff])).max() > 0)
    has_o_bias = bool(np.abs(bo).max() > 0)
    has_proj_bias = bool(np.abs(bproj).max() > 0)
    has_fc_bias = bool(np.abs(bfc_eff).max() > 0)
    has_mask = bool(padding_mask.any())

    nc = _get_program((has_qkv_bias, has_o_bias, has_proj_bias, has_fc_bias, has_mask))

    shared = {
        "wq": wq_eff, "wk": wk_eff, "wv": wv_eff, "wo": wo_pre,
        "wfc": wfc_pre, "wproj": wproj_pre, "wsumsfc": wsumsfc,
        "biases": bias_pre, "bias_rows": brows,
    }
    in_maps = []
    for c in range(NCORES):
        b, qo = c // (NCORES // B), (c % (NCORES // B)) * TQ
        xr = np.roll(x[b], -qo, axis=0)
        x_fm = np.ascontiguousarray(xr.T)
        x_bf = x_fm.astype(ml_dtypes.bfloat16)
        mrow = np.roll(padding_mask[b], -qo)
        maskb = np.ascontiguousarray(
            np.where(mrow, -1e30, 0.0).astype(np.float32).reshape(NSC, 128).T)
        in_maps.append({**shared, "x_fm": x_fm, "x_bf": x_bf, "maskb": maskb})

    res = run_bass_kernel_spmd(nc, in_maps, core_ids=list(range(NCORES)))

    out = np.empty((B, T, C), dtype=np.float32)
    for c in range(NCORES):
        b, qo = c // (NCORES // B), (c % (NCORES // B)) * TQ
        out[b, qo:qo + TQ, :] = res.results[c]["out_fm"].T
    return out


# revision 15
# speedup vs baseline: 1.3010x; 1.0983x over previous
"""Trainium2 Bass kernel for a GPT-style transformer block (B=2,T=2048,C=768,H=12).

Sharding: 8 cores; core c handles batch b=c//4, query block qo=(c%4)*512.
Each core gets its batch's x feature-major [C,T], rolled so its 512 query
tokens are columns 0:512.  K/V are computed for all 2048 keys (duplicated
across the 4 cores of a batch); Q/attention/MLP only for the 512 queries.

v2 structure (vs the 351us baseline):
 - x_bf is DMA'd in four 512-token chunks and LN1 stats / V / K0 / Q are
   pipelined per chunk, so the PE starts ~6us in instead of ~30us.
 - Scores are row-tiled: head pair (2ch, 2ch+1) runs as two concurrent
   K=64 matmuls (tile_position (0,0)/(64,0)) into two PSUM banks, halving
   score time.  One exp covers both banks.  Denominators still ride the
   augmented-V ones column.
 - Softmax reciprocal stays on DVE except the last pair (Act ln/exp) so
   Wo isn't blocked by the 3.4us DVE reciprocal.
 - LN2 is folded into the FC matmul: FC runs on the *uncentered* residual
   x2 (bf16) immediately after Wo, with a rank-1 (-mu * colsum(Wfc))
   correction and the 1/std multiply applied at PSUM copyback before
   gelu.  The PE never idles (and never goes HAM-cold) at the Wo->MLP
   boundary.
 - MLP is two passes: FC -> hc (all 24 chunks in SBUF), then proj
   oc-major with per-oc output DMA, so the output store overlaps the
   last proj matmuls.
 - Act only ever uses the natural_log_exp set until the first gelu
   (squares moved to DVE/GpSimd), so ~2 table loads instead of 12.
"""
import sys

sys.path.insert(0, "/opt/trn_rl_repo")

import numpy as np
import ml_dtypes

import concourse.bass as bass
import concourse.tile as tile
from concourse import bacc, mybir
from concourse.bass_utils import run_bass_kernel_spmd

F32 = mybir.dt.float32
F32R = mybir.dt.float32r
BF16 = mybir.dt.bfloat16
AF = mybir.ActivationFunctionType
ALU = mybir.AluOpType

B, T, C, H = 2, 2048, 768, 12
HD = C // H             # 64
C4 = 4 * C              # 3072
EPS = 1e-5
NCORES = 8
TQ = (B * T) // NCORES  # 512
PC = C // 128           # 6
PC4 = C4 // 128         # 24
NT4 = T // 512          # 4
NSC = T // 128          # 16
NBIAS = (5 * C + C4) // 128  # 54
NPAIR = H // 2          # 6
FC_DELAY = 5            # kc lag between FC accumulate and gelu copyback


def _build(has_qkv_bias, has_o_bias, has_proj_bias, has_fc_bias, has_mask, reps=1):
    has_bias_any = has_qkv_bias or has_o_bias or has_proj_bias or has_fc_bias
    nc = bacc.Bacc()

    x_d = nc.dram_tensor("x_fm", [C, T], F32, kind="ExternalInput")
    xb_d = nc.dram_tensor("x_bf", [C, T], BF16, kind="ExternalInput")
    wq_d = nc.dram_tensor("wq", [C, C], BF16, kind="ExternalInput")
    wk_d = nc.dram_tensor("wk", [C, C], BF16, kind="ExternalInput")
    wv_d = nc.dram_tensor("wv", [C, C], BF16, kind="ExternalInput")
    wo_d = nc.dram_tensor("wo", [128, PC, C], BF16, kind="ExternalInput")
    wfc_d = nc.dram_tensor("wfc", [PC4, 128, PC, 128], BF16, kind="ExternalInput")
    wproj_d = nc.dram_tensor("wproj", [PC, 128, PC4, 128], BF16, kind="ExternalInput")
    wsumfc_d = nc.dram_tensor("wsumsfc", [1, C4], BF16, kind="ExternalInput")
    bias_d = nc.dram_tensor("biases", [128, NBIAS], F32, kind="ExternalInput")
    brow_d = nc.dram_tensor("bias_rows", [1, 3 * C], BF16, kind="ExternalInput")
    mask_d = nc.dram_tensor("maskb", [128, NSC], F32, kind="ExternalInput")
    out_d = nc.dram_tensor("out_fm", [C, TQ], F32, kind="ExternalOutput")

    x_pot = x_d.rearrange("(o p) t -> p o t", p=128)
    xb_pot = xb_d.rearrange("(o p) t -> p o t", p=128)
    out_pot = out_d.rearrange("(o p) t -> p o t", p=128)

    with tile.TileContext(nc) as tc:
      for _rep in range(reps):
        with tc.tile_pool(name=f"const{_rep}", bufs=1) as const, \
             tc.tile_pool(name=f"persist{_rep}", bufs=1) as persist, \
             tc.tile_pool(name=f"mlp1_{_rep}", bufs=1) as mlp1:

            # ---------------- constants (no DMA) ----------------
            ones_col_b = const.tile([128, 1], BF16)
            nc.vector.memset(ones_col_b[:], 1.0)
            ones_row_b = const.tile([1, 128], BF16)
            nc.vector.memset(ones_row_b[:], 1.0)
            ones_mat = const.tile([128, HD + 1], BF16)
            nc.vector.memset(ones_mat[:], 1.0)

            # residual tile: holds x for the queries, then x + attn_out in place
            x2 = persist.tile([128, PC, TQ], F32)
            x2b = mlp1.tile([128, PC, TQ], BF16)
            hc_all = mlp1.tile([128, PC4, TQ], BF16)
            wsumfc_sb = mlp1.tile([1, C4], BF16)

            with tc.tile_pool(name=f"ypool{_rep}", bufs=1) as ypool:
              y_sb = ypool.tile([HD + 1, H, TQ], BF16)
              y_nm2 = ypool.tile([128, PC, TQ], BF16)
              wo_sb = ypool.tile([128, PC, C], BF16)
              with tc.tile_pool(name=f"attp{_rep}", bufs=1) as attp:
                q2 = attp.tile([128, NPAIR, TQ], BF16)
                k_bf = attp.tile([128, PC, T], BF16)
                vt_aug = attp.tile([128, NSC, H * (HD + 1)], BF16)
                x_bf = attp.tile([128, PC, T], BF16)
                istd_b = attp.tile([128, T], BF16)
                istd_col = attp.tile([128, NSC], F32)
                risd_r = (attp.tile([1, T], BF16)    # sqrt(var+eps) (bias path)
                          if has_qkv_bias else None)
                wk_sb = attp.tile([128, PC, C], BF16)

                # ======== phase A: LN1 stats + V + K0 + Q, per chunk ========
                with tc.tile_pool(name=f"rtmp{_rep}", bufs=2) as rtmp, \
                     tc.tile_pool(name=f"gsc{_rep}", bufs=2) as gsc, \
                     tc.tile_pool(name=f"wcyc{_rep}", bufs=2) as wcyc, \
                     tc.tile_pool(name=f"st_ps{_rep}", bufs=1, space="PSUM") as st_ps, \
                     tc.tile_pool(name=f"p1_ps{_rep}", bufs=2, space="PSUM") as p1_ps, \
                     tc.tile_pool(name=f"vq_ps{_rep}", bufs=2, space="PSUM") as vq_ps:

                    # ---- DMAs in priority order ----
                    nc.sync.dma_start(x_bf[:, :, 0:512], xb_pot[:, :, 0:512])
                    wv_sb = wcyc.tile([128, PC, C], BF16, tag="w")
                    nc.sync.dma_start(wv_sb[:], wv_d.rearrange("(o p) m -> p o m", p=128))
                    for t4 in range(1, NT4):
                        sl = slice(t4 * 512, (t4 + 1) * 512)
                        nc.sync.dma_start(x_bf[:, :, sl], xb_pot[:, :, sl])
                    wq_sb = wcyc.tile([128, PC, C], BF16, tag="w")
                    nc.sync.dma_start(wq_sb[:], wq_d.rearrange("(o p) m -> p o m", p=128))
                    nc.sync.dma_start(wk_sb[:], wk_d.rearrange("(o p) m -> p o m", p=128))
                    nc.sync.dma_start(wsumfc_sb[:], wsumfc_d[:, :])
                    if has_bias_any:
                        bias_sb = const.tile([128, NBIAS], F32)
                        nc.sync.dma_start(bias_sb[:], bias_d[:, :])
                    if has_mask:
                        mask_sb = const.tile([128, NSC], F32)
                        nc.sync.dma_start(mask_sb[:], mask_d[:, :])
                    if has_qkv_bias:
                        brow_sb = const.tile([1, 3 * C], BF16)
                        nc.sync.dma_start(brow_sb[:], brow_d[:, :])

                    p1s = {}

                    def emit_p1(t4):
                        sl = slice(t4 * 512, (t4 + 1) * 512)
                        p1 = p1_ps.tile([1, 512], F32, tag="p1")
                        for j in range(PC):
                            nc.tensor.matmul(p1[:], ones_col_b[:], x_bf[:, j, sl],
                                             start=(j == 0), stop=(j == PC - 1))
                        p1s[t4] = p1

                    def emit_center(t4):
                        # negmu row -> broadcast -> center x_bf in place
                        sl = slice(t4 * 512, (t4 + 1) * 512)
                        negmu_c = rtmp.tile([1, 512], BF16, tag="rtb")
                        nc.vector.tensor_scalar_mul(negmu_c[:], p1s[t4][:], -1.0 / C)
                        nm_ps = st_ps.tile([128, 512], F32, tag="nm")
                        nc.tensor.matmul(nm_ps[:], ones_row_b[:], negmu_c[:],
                                         start=True, stop=True)
                        nm_sb = gsc.tile([128, 512], BF16, tag="nmsb")
                        nc.scalar.activation(nm_sb[:], nm_ps[:], AF.Copy)
                        for j in range(PC):
                            eng = nc.vector if j < 3 else nc.gpsimd
                            eng.tensor_tensor(x_bf[:, j, sl], x_bf[:, j, sl],
                                              nm_sb[:], ALU.add)

                    def emit_var_chain(t4):
                        # squares (DVE/GpSimd only) -> p2 -> istd row/col/broadcast
                        sl = slice(t4 * 512, (t4 + 1) * 512)
                        p2 = p1_ps.tile([1, 512], F32, tag="p2")
                        for j in range(PC):
                            xsq = gsc.tile([128, 512], BF16, tag="xsq")
                            if j < 3:
                                nc.scalar.activation(xsq[:], x_bf[:, j, sl], AF.Square)
                            else:
                                eng = nc.vector if j < 5 else nc.gpsimd
                                eng.tensor_tensor(xsq[:], x_bf[:, j, sl], x_bf[:, j, sl],
                                                  ALU.mult)
                            nc.tensor.matmul(p2[:], ones_col_b[:], xsq[:],
                                             start=(j == 0), stop=(j == PC - 1))
                        var_c = rtmp.tile([1, 512], F32, tag="rt")
                        nc.vector.tensor_scalar(var_c[:], p2[:], 1.0 / C, EPS,
                                                ALU.mult, ALU.add)
                        # sqrt on Act (single table set); reciprocal on DVE in
                        # column form (recip cost scales with the free dim, so
                        # [128,4] is ~free where a [1,512] row costs 3.4us)
                        rsd_c = rtmp.tile([1, 512], F32, tag="rt")
                        nc.scalar.activation(rsd_c[:], var_c[:], AF.Sqrt)
                        if has_qkv_bias:
                            nc.vector.tensor_copy(risd_r[:, sl], rsd_c[:])
                        rsd_col = rtmp.tile([128, 4], F32, tag="rtc")
                        for o in range(4):
                            nc.sync.dma_start(rsd_col[:, o:o + 1],
                                              rsd_c[0:1, o * 128:(o + 1) * 128])
                        nc.vector.reciprocal(istd_col[:, t4 * 4:t4 * 4 + 4], rsd_col[:])
                        istd_c = rtmp.tile([1, 512], F32, tag="rt")
                        for o in range(4):
                            nc.sync.dma_start(istd_c[0:1, o * 128:(o + 1) * 128],
                                              istd_col[:, t4 * 4 + o:t4 * 4 + o + 1])
                        istd_cb = rtmp.tile([1, 512], BF16, tag="rtb")
                        nc.vector.tensor_copy(istd_cb[:], istd_c[:])
                        bp = st_ps.tile([128, 512], F32, tag="bp")
                        nc.tensor.matmul(bp[:], ones_row_b[:], istd_cb[:],
                                         start=True, stop=True)
                        nc.scalar.activation(istd_b[:, sl], bp[:], AF.Copy)

                    def emit_v(t4):
                        # V for the 4 key blocks of this chunk (token-major, aug)
                        for sc in range(4 * t4, 4 * t4 + 4):
                            ssl = slice(sc * 128, (sc + 1) * 128)
                            nc.gpsimd.memset(
                                vt_aug[:, sc, :].rearrange("p (h e) -> p h e", e=HD + 1)[:, :, HD:HD + 1],
                                1.0)
                            for half in range(2):
                                hsl = slice(half * 384, (half + 1) * 384)
                                csl = slice(2 * C + half * 384, 2 * C + (half + 1) * 384)
                                vp = vq_ps.tile([128, 512], F32, tag="pp", name="vp")[:, 0:384]
                                for j in range(PC):
                                    nc.tensor.matmul(vp[:], x_bf[:, j, ssl], wv_sb[:, j, hsl],
                                                     start=(j == 0),
                                                     stop=(j == PC - 1 and not has_qkv_bias))
                                if has_qkv_bias:
                                    nc.tensor.matmul(vp[:], risd_r[:, ssl], brow_sb[:, csl],
                                                     start=False, stop=True)
                                dst = vt_aug[:, sc, :].rearrange("p (h e) -> p h e", e=HD + 1)[
                                    :, half * 6:(half + 1) * 6, 0:HD]
                                if half == 0:
                                    nc.scalar.activation(
                                        dst, vp[:].rearrange("p (h e) -> p h e", e=HD),
                                        AF.Copy, scale=istd_col[:, sc:sc + 1])
                                else:
                                    nc.vector.tensor_scalar(
                                        dst, vp[:].rearrange("p (h e) -> p h e", e=HD),
                                        istd_col[:, sc:sc + 1], None, ALU.mult)

                    def emit_k0(t4):
                        sl = slice(t4 * 512, (t4 + 1) * 512)
                        kp = vq_ps.tile([128, 512], F32, tag="pp", name="kp")
                        for j in range(PC):
                            nc.tensor.matmul(kp[:], wk_sb[:, j, 0:128],
                                             x_bf[:, j, sl], start=(j == 0),
                                             stop=(j == PC - 1 and not has_qkv_bias))
                        if has_qkv_bias:
                            nc.tensor.matmul(kp[:], brow_sb[:, C:C + 128],
                                             risd_r[:, sl], start=False, stop=True)
                        nc.vector.tensor_tensor(k_bf[:, 0, sl], kp[:], istd_b[:, sl],
                                                ALU.mult)

                    def emit_q():
                        # queries only (chunk 0); both heads of a pair in one tile
                        for oc in range(PC):
                            osl = slice(oc * 128, (oc + 1) * 128)
                            qp = vq_ps.tile([128, 512], F32, tag="pp", name="qp")
                            for j in range(PC):
                                nc.tensor.matmul(qp[:], wq_sb[:, j, osl],
                                                 x_bf[:, j, 0:TQ], start=(j == 0),
                                                 stop=(j == PC - 1 and not has_qkv_bias))
                            if has_qkv_bias:
                                nc.tensor.matmul(qp[:], brow_sb[:, osl],
                                                 risd_r[:, 0:TQ], start=False, stop=True)
                            nc.vector.tensor_tensor(q2[:, oc, :], qp[:],
                                                    istd_b[:, 0:TQ], ALU.mult)

                    # stats chains for all chunks first (istd ready early),
                    # then the V/K0/Q bulk streams with no PSUM backpressure
                    emit_p1(0)
                    emit_center(0)
                    emit_p1(1)
                    emit_var_chain(0)
                    emit_center(1)
                    emit_p1(2)
                    emit_var_chain(1)
                    emit_center(2)
                    emit_p1(3)
                    emit_var_chain(2)
                    emit_center(3)
                    emit_var_chain(3)
                    # residual / Wo loads go behind the stats-chain DMAs
                    nc.sync.dma_start(x2[:], x_pot[:, :, 0:TQ])
                    nc.sync.dma_start(wo_sb[:], wo_d[:, :, :])
                    emit_v(0)
                    emit_k0(0)
                    emit_q()
                    emit_v(1)
                    emit_k0(1)
                    emit_v(2)
                    emit_k0(2)
                    emit_v(3)
                    emit_k0(3)

                # ============ phase B: attention (K oc=1..5 interleaved) ====
                with tc.tile_pool(name=f"sc_ps{_rep}", bufs=2, space="PSUM") as sc_ps, \
                     tc.tile_pool(name=f"y_psp{_rep}", bufs=2, space="PSUM") as y_psp, \
                     tc.tile_pool(name=f"rp_ps{_rep}", bufs=1, space="PSUM") as rp_ps, \
                     tc.tile_pool(name=f"kp_ps{_rep}", bufs=1, space="PSUM") as kp_ps, \
                     tc.tile_pool(name=f"attb{_rep}", bufs=3) as attb, \
                     tc.tile_pool(name=f"recb{_rep}", bufs=2) as recb:

                    def k_chunk_gen(oc):
                        # yields after each PE matmul; copybacks on DVE
                        osl = slice(oc * 128, (oc + 1) * 128)
                        for t4 in range(NT4):
                            sl = slice(t4 * 512, (t4 + 1) * 512)
                            kp = kp_ps.tile([128, 512], F32, tag="kp")
                            for j in range(PC):
                                nc.tensor.matmul(
                                    kp[:], wk_sb[:, j, osl],
                                    x_bf[:, j, sl], start=(j == 0),
                                    stop=(j == PC - 1 and not has_qkv_bias))
                                if j < PC - 1:
                                    yield
                            if has_qkv_bias:
                                nc.tensor.matmul(
                                    kp[:], brow_sb[:, C + oc * 128:C + (oc + 1) * 128],
                                    risd_r[:, sl], start=False, stop=True)
                            nc.vector.tensor_tensor(k_bf[:, oc, sl], kp[:],
                                                    istd_b[:, sl], ALU.mult)
                            yield

                    def make_tail(ch, yps):
                        def head_tail(h, yp):
                            nc.vector.tensor_copy(y_sb[:, h, :], yp[:])
                            # reciprocal of the denominator row in column form
                            # (DVE recip cost scales with the free dim)
                            d_col = recb.tile([128, 4], F32, tag="dcol")
                            for o in range(4):
                                nc.sync.dma_start(d_col[:, o:o + 1],
                                                  y_sb[HD:HD + 1, h, o * 128:(o + 1) * 128])
                            r_col = recb.tile([128, 4], F32, tag="rcol")
                            nc.vector.reciprocal(r_col[:], d_col[:])
                            rrow = recb.tile([1, TQ], F32, tag="rrow")
                            for o in range(4):
                                nc.sync.dma_start(rrow[0:1, o * 128:(o + 1) * 128],
                                                  r_col[:, o:o + 1])
                            rp = rp_ps.tile([HD + 1, TQ], F32, tag="rp")
                            nc.tensor.matmul(rp[:], ones_mat[0:1, 0:HD + 1],
                                             rrow[:], start=True, stop=True)
                            rec = recb.tile([HD + 1, TQ], F32, tag="rec")
                            nc.vector.tensor_copy(rec[:], rp[:])
                            if h % 2 == 0:
                                nc.gpsimd.tensor_tensor(y_nm2[0:HD, h // 2, :],
                                                        y_sb[0:HD, h, :],
                                                        rec[0:HD, :], ALU.mult)
                            else:
                                ytmp = recb.tile([HD, TQ], BF16, tag="ytmp")
                                nc.gpsimd.tensor_tensor(ytmp[:], y_sb[0:HD, h, :],
                                                        rec[0:HD, :], ALU.mult)
                                nc.sync.dma_start(y_nm2[HD:128, h // 2, :], ytmp[:])

                        def tail():
                            head_tail(2 * ch, yps[0])
                            head_tail(2 * ch + 1, yps[1])
                        return tail

                    kgen = None
                    pending_tail = None
                    for ch in range(NPAIR):
                        if ch < NPAIR - 1:
                            kgen = k_chunk_gen(ch + 1)
                        yp_a = y_psp.tile([HD + 1, TQ], F32, tag="yp")
                        yp_b = y_psp.tile([HD + 1, TQ], F32, tag="yp")
                        prev_av = None
                        for sc in range(NSC):
                            sp = sc_ps.tile([128, 2, 512], F32, tag="sp")
                            nc.tensor.matmul(sp[:, 0, :],
                                             k_bf[0:64, ch, sc * 128:(sc + 1) * 128],
                                             q2[0:64, ch, :],
                                             start=True, stop=True,
                                             tile_position=(0, 0))
                            nc.tensor.matmul(sp[:, 1, :],
                                             k_bf[64:128, ch, sc * 128:(sc + 1) * 128],
                                             q2[64:128, ch, :],
                                             start=True, stop=True,
                                             tile_position=(64, 0))
                            att = attb.tile([128, 2, 512], BF16, tag="att")
                            if has_mask:
                                for i in range(2):
                                    nc.scalar.activation(att[:, i, :], sp[:, i, :], AF.Exp,
                                                         bias=mask_sb[:, sc:sc + 1])
                            else:
                                nc.scalar.activation(att[:], sp[:], AF.Exp)
                            if prev_av is not None:
                                prev_av()
                            if pending_tail is not None:
                                pending_tail()
                                pending_tail = None
                            if kgen is not None:
                                for _ in range(2 if sc % 2 == 0 else 1):
                                    if next(kgen, "end") == "end":
                                        kgen = None
                                        break

                            def av(att=att, sc=sc, ch=ch, yp_a=yp_a, yp_b=yp_b):
                                nc.tensor.matmul(yp_a[:],
                                                 vt_aug[:, sc, 65 * 2 * ch:65 * 2 * ch + 65],
                                                 att[:, 0, :],
                                                 start=(sc == 0), stop=(sc == NSC - 1))
                                nc.tensor.matmul(yp_b[:],
                                                 vt_aug[:, sc, 65 * (2 * ch + 1):65 * (2 * ch + 1) + 65],
                                                 att[:, 1, :],
                                                 start=(sc == 0), stop=(sc == NSC - 1))
                            prev_av = av
                        prev_av()
                        pending_tail = make_tail(ch, (yp_a, yp_b))
                        if ch == NPAIR - 1:
                            # preload the sqrt table set during the tail/Wo window
                            warm = recb.tile([1, 1], F32, tag="warm")
                            nc.scalar.activation(warm[:], ones_mat[0:1, 0:1], AF.Sqrt)
                    pending_tail()

              # ---- phase C: Wo (pairs 0-4 first) + LN2 stats interleaved ----
              negmu2_r = mlp1.tile([1, TQ], BF16)
              istd2_cb = mlp1.tile([1, TQ], BF16)
              istd2_b = mlp1.tile([128, TQ], BF16)
              with tc.tile_pool(name=f"dtmp{_rep}", bufs=2) as dtmp:
                with tc.tile_pool(name=f"wo_ps{_rep}", bufs=1, space="PSUM") as wo_ps, \
                     tc.tile_pool(name=f"xsqp{_rep}", bufs=2) as xsqp, \
                     tc.tile_pool(name=f"d_ps{_rep}", bufs=1, space="PSUM") as d_ps:
                  p1 = d_ps.tile([1, TQ], F32, tag="p1")
                  p2 = d_ps.tile([1, TQ], F32, tag="p2")
                  wops = []
                  for oc in range(PC):
                      op = wo_ps.tile([128, TQ], F32, tag=f"op{oc}", name=f"op{oc}")
                      for hp in range(PC - 1):
                          nc.tensor.matmul(op[:], wo_sb[:, hp, oc * 128:(oc + 1) * 128],
                                           y_nm2[:, hp, :], start=(hp == 0), stop=False)
                      wops.append(op)
                  for oc in range(PC):
                      nc.tensor.matmul(wops[oc][:], wo_sb[:, PC - 1, oc * 128:(oc + 1) * 128],
                                       y_nm2[:, PC - 1, :], start=False, stop=True)
                      op = wops[oc]
                      if has_o_bias:
                          nc.scalar.activation(op[:], op[:], AF.Identity,
                                               bias=bias_sb[:, 3 * PC + oc:3 * PC + oc + 1])
                      nc.vector.tensor_tensor(x2[:, oc, :], x2[:, oc, :], op[:],
                                              ALU.add)
                      nc.vector.tensor_copy(x2b[:, oc, :], x2[:, oc, :])
                      xsqa = xsqp.tile([128, TQ], BF16, tag="xsqa")
                      nc.gpsimd.tensor_tensor(xsqa[:], x2b[:, oc, :], x2b[:, oc, :],
                                              ALU.mult)
                      nc.tensor.matmul(p1[:], ones_col_b[:], x2b[:, oc, :],
                                       start=(oc == 0), stop=(oc == PC - 1))
                      nc.tensor.matmul(p2[:], ones_col_b[:], xsqa[:],
                                       start=(oc == 0), stop=(oc == PC - 1))

                  # LN2 scalars that read p1/p2 (before d_ps closes)
                  nc.vector.tensor_scalar_mul(negmu2_r[:], p1[:], -1.0 / C)
                  msq2 = dtmp.tile([1, TQ], F32, tag="dt")
                  nc.vector.tensor_tensor(msq2[:], negmu2_r[:], negmu2_r[:], ALU.mult)
                  var2 = dtmp.tile([1, TQ], F32, tag="dt")
                  nc.vector.tensor_scalar(var2[:], p2[:], 1.0 / C, EPS,
                                          ALU.mult, ALU.add)
                  nc.vector.tensor_sub(var2[:], var2[:], msq2[:])

                rsd2 = dtmp.tile([1, TQ], F32, tag="dt")
                nc.scalar.activation(rsd2[:], var2[:], AF.Sqrt)
                warm2 = dtmp.tile([1, 1], F32, tag="dtw")
                nc.scalar.activation(warm2[:], rsd2[0:1, 0:1], AF.Gelu)
                rsd2_col = dtmp.tile([128, 4], F32, tag="dtc")
                for o in range(4):
                    nc.sync.dma_start(rsd2_col[:, o:o + 1],
                                      rsd2[0:1, o * 128:(o + 1) * 128])
                istd2_col = dtmp.tile([128, 4], F32, tag="dtc")
                nc.vector.reciprocal(istd2_col[:], rsd2_col[:])
                istd2 = dtmp.tile([1, TQ], F32, tag="dt")
                for o in range(4):
                    nc.sync.dma_start(istd2[0:1, o * 128:(o + 1) * 128],
                                      istd2_col[:, o:o + 1])
                nc.vector.tensor_copy(istd2_cb[:], istd2[:])

            # ============ phase D: MLP (LN2 folded into FC) ============
            with tc.tile_pool(name=f"wpo{_rep}", bufs=2) as wpo_pool:
              wpos = []

              def prefetch_wpo():
                  wpo = wpo_pool.tile([128, PC4, 128], BF16, tag="wpo")
                  nc.sync.dma_start(wpo[:], wproj_d[len(wpos)])
                  wpos.append(wpo)

              with tc.tile_pool(name=f"b2_ps{_rep}", bufs=1, space="PSUM") as b2_ps, \
                   tc.tile_pool(name=f"fc_ps{_rep}", bufs=FC_DELAY + 2, space="PSUM") as fc_ps, \
                   tc.tile_pool(name=f"h_sb{_rep}", bufs=2) as h_sb, \
                   tc.tile_pool(name=f"w_sb2{_rep}", bufs=3) as w_sb2:
                    bp2 = b2_ps.tile([128, TQ], F32, tag="bp2")
                    nc.tensor.matmul(bp2[:], ones_row_b[:], istd2_cb[:],
                                     start=True, stop=True)
                    nc.scalar.activation(istd2_b[:], bp2[:], AF.Copy)

                    fps = {}

                    def finish_kc(kc):
                        fp = fps.pop(kc)
                        # rank-1 LN2 mean fold, then istd multiply + gelu
                        nc.tensor.matmul(fp[:], wsumfc_sb[:, kc * 128:(kc + 1) * 128],
                                         negmu2_r[:], start=False, stop=True)
                        hm = h_sb.tile([128, TQ], BF16, tag="hm")
                        nc.vector.tensor_tensor(hm[:], fp[:], istd2_b[:], ALU.mult)
                        if has_fc_bias:
                            nc.scalar.activation(hc_all[:, kc, :], hm[:], AF.Gelu,
                                                 bias=bias_sb[:, 5 * PC + kc:5 * PC + kc + 1])
                        else:
                            nc.scalar.activation(hc_all[:, kc, :], hm[:], AF.Gelu)

                    for kc in range(PC4):
                        wfcc = w_sb2.tile([128, PC, 128], BF16, tag="wfcc")
                        nc.sync.dma_start(wfcc[:], wfc_d[kc])
                        fp = fc_ps.tile([128, TQ], F32, tag="fp")
                        for j in range(PC):
                            nc.tensor.matmul(fp[:], wfcc[:, j, :], x2b[:, j, :],
                                             start=(j == 0), stop=False)
                        fps[kc] = fp
                        if kc >= FC_DELAY:
                            finish_kc(kc - FC_DELAY)
                        if kc in (16, 20):
                            prefetch_wpo()
                    for kc in range(PC4 - FC_DELAY, PC4):
                        finish_kc(kc)

              # ---- proj, oc-major, with per-oc output DMA ----
              with tc.tile_pool(name=f"pr_ps{_rep}", bufs=2, space="PSUM") as pr_ps, \
                   tc.tile_pool(name=f"outp{_rep}", bufs=2) as outp:
                    for oc in range(PC):
                        if len(wpos) < PC:
                            prefetch_wpo()
                        wpo = wpos[oc]
                        pr = pr_ps.tile([128, TQ], F32, tag="pr")
                        for kc in range(PC4):
                            nc.tensor.matmul(pr[:], wpo[:, kc, :], hc_all[:, kc, :],
                                             start=(kc == 0), stop=(kc == PC4 - 1))
                        if has_proj_bias:
                            nc.scalar.activation(pr[:], pr[:], AF.Identity,
                                                 bias=bias_sb[:, 4 * PC + oc:4 * PC + oc + 1])
                        out_t = outp.tile([128, TQ], F32, tag="out")
                        nc.vector.tensor_tensor(out_t[:], pr[:], x2[:, oc, :],
                                                ALU.add)
                        nc.sync.dma_start(out_pot[:, oc, :], out_t[:])

    nc.compile()
    return nc


_CACHE = {}


def _get_program(flags, reps=1):
    key = (flags, reps)
    if key not in _CACHE:
        _CACHE[key] = _build(*flags, reps=reps)
    return _CACHE[key]


def kernel(**inputs) -> np.ndarray:
    x = np.asarray(inputs["x"], dtype=np.float32)
    padding_mask = np.asarray(inputs["padding_mask"])
    ln1_s = np.asarray(inputs["ln1_scale"], dtype=np.float32)
    ln1_b = np.asarray(inputs["ln1_bias"], dtype=np.float32)
    ln2_s = np.asarray(inputs["ln2_scale"], dtype=np.float32)
    ln2_b = np.asarray(inputs["ln2_bias"], dtype=np.float32)
    Wq = np.asarray(inputs["Wq"], dtype=np.float32)
    Wk = np.asarray(inputs["Wk"], dtype=np.float32)
    Wv = np.asarray(inputs["Wv"], dtype=np.float32)
    bq = np.asarray(inputs["bq"], dtype=np.float32)
    bk = np.asarray(inputs["bk"], dtype=np.float32)
    bv = np.asarray(inputs["bv"], dtype=np.float32)
    Wo = np.asarray(inputs["Wo"], dtype=np.float32)
    bo = np.asarray(inputs["bo"], dtype=np.float32)
    Wfc = np.asarray(inputs["Wfc"], dtype=np.float32)
    bfc = np.asarray(inputs["bfc"], dtype=np.float32)
    Wproj = np.asarray(inputs["Wproj"], dtype=np.float32)
    bproj = np.asarray(inputs["bproj"], dtype=np.float32)

    sc_q = 1.0 / np.sqrt(HD)
    Wq_f = Wq.transpose(1, 0, 2).reshape(C, C)
    Wk_f = Wk.transpose(1, 0, 2).reshape(C, C)
    Wv_f = Wv.transpose(1, 0, 2).reshape(C, C)
    wq_eff = (ln1_s[:, None] * Wq_f * sc_q).astype(ml_dtypes.bfloat16)
    wk_eff = (ln1_s[:, None] * Wk_f).astype(ml_dtypes.bfloat16)
    wv_eff = (ln1_s[:, None] * Wv_f).astype(ml_dtypes.bfloat16)
    bq_eff = (ln1_b @ Wq_f) * sc_q + bq.reshape(C) * sc_q
    bk_eff = ln1_b @ Wk_f + bk.reshape(C)
    bv_eff = ln1_b @ Wv_f + bv.reshape(C)
    wfc_eff = (ln2_s[:, None] * Wfc).astype(ml_dtypes.bfloat16)
    bfc_eff = ln2_b @ Wfc + bfc
    wfc_pre = np.ascontiguousarray(
        wfc_eff.reshape(PC, 128, PC4, 128).transpose(2, 1, 0, 3))
    wproj_pre = np.ascontiguousarray(
        Wproj.reshape(PC4, 128, PC, 128).transpose(2, 1, 0, 3)).astype(ml_dtypes.bfloat16)
    wo_pre = np.ascontiguousarray(
        Wo.reshape(PC, 2, HD, C).transpose(1, 2, 0, 3).reshape(128, PC, C)
    ).astype(ml_dtypes.bfloat16)

    wsumsfc = wfc_eff.astype(np.float32).sum(0).astype(ml_dtypes.bfloat16)[None, :]

    biases = np.concatenate([bq_eff, bk_eff, bv_eff, bo, bproj, bfc_eff])
    bias_pre = np.ascontiguousarray(biases.reshape(NBIAS, 128).T).astype(np.float32)
    brows = np.concatenate([bq_eff, bk_eff, bv_eff]).astype(ml_dtypes.bfloat16)[None, :]

    has_qkv_bias = bool(np.abs(np.concatenate([bq_eff, bk_eff, bv_eff])).max() > 0)
    has_o_bias = bool(np.abs(bo).max() > 0)
    has_proj_bias = bool(np.abs(bproj).max() > 0)
    has_fc_bias = bool(np.abs(bfc_eff).max() > 0)
    has_mask = bool(padding_mask.any())

    nc = _get_program((has_qkv_bias, has_o_bias, has_proj_bias, has_fc_bias, has_mask))

    shared = {
        "wq": wq_eff, "wk": wk_eff, "wv": wv_eff, "wo": wo_pre,
        "wfc": wfc_pre, "wproj": wproj_pre, "wsumsfc": wsumsfc,
        "biases": bias_pre, "bias_rows": brows,
    }
    in_maps = []
    for c in range(NCORES):
        b, qo = c // (NCORES // B), (c % (NCORES // B)) * TQ
        xr = np.roll(x[b], -qo, axis=0)
        x_fm = np.ascontiguousarray(xr.T)
        x_bf = x_fm.astype(ml_dtypes.bfloat16)
        mrow = np.roll(padding_mask[b], -qo)
        maskb = np.ascontiguousarray(
            np.where(mrow, -1e30, 0.0).astype(np.float32).reshape(NSC, 128).T)
        in_maps.append({**shared, "x_fm": x_fm, "x_bf": x_bf, "maskb": maskb})

    res = run_bass_kernel_spmd(nc, in_maps, core_ids=list(range(NCORES)))

    out = np.empty((B, T, C), dtype=np.float32)
    for c in range(NCORES):
        b, qo = c // (NCORES // B), (c % (NCORES // B)) * TQ
        out[b, qo:qo + TQ, :] = res.results[c]["out_fm"].T
    return out
